# revision 1
# baseline (speedup 1.0000x reference)
"""Trainium2 Bass kernel for nn_Attention4D (EfficientViT-style attention).

Strategy (8 NeuronCores, data-parallel over batch B=8, one batch element per core):
  - BN folded into conv weights on host.
  - Talking-head-1 folded into per-head-scaled queries (Q2), so q@k contracts
    256 channels at full PE efficiency and th1 disappears.
  - Attention tensor layout: partition dim = (head, 16 queries) interleaved
    (49 groups of 128 partitions), free dim = keys m (784).  Softmax is then
    per-partition (ACT exp with fused accumulate for the denominator; bth1
    rides the free per-partition ACT bias), and talking-head-2 is one
    block-diagonal 128x128 PE matmul per group with the softmax normalization
    folded into the block-diag weights.
  - Relative-position biases are a precomputed fp8 table accumulated into the
    logits PSUM with an identity matmul.
  - attn@v needs keys on partitions, so A2 makes one DRAM round trip through
    dma_start_transpose (two scratch tensors so the read-back overlaps the
    tail of the softmax phase).  V^T is produced directly by the projection.
  - The 3x3 depthwise conv (v_local) runs on the PE as 9 diagonal-weight
    matmuls accumulated into the same PSUM banks as attn@v output.
"""

import sys

sys.path.insert(0, "/opt/trn_rl_repo")

import numpy as np
import ml_dtypes

import concourse.bass as bass
import concourse.tile as tile
from concourse import bacc, mybir
from concourse.bass_utils import run_bass_kernel_spmd

F32 = mybir.dt.float32
BF16 = mybir.dt.bfloat16
FP8 = mybir.dt.float8e4
AF = mybir.ActivationFunctionType
BF = ml_dtypes.bfloat16
F8 = ml_dtypes.float8_e4m3

HEADS, KD, AR, RES, DIM = 8, 32, 4, 28, 384
D = AR * KD            # 128
DH = HEADS * D         # 1024
NH_KD = HEADS * KD     # 256
N = RES * RES          # 784
NG = N // 16           # 49 groups of 16 queries
NGA = 28               # groups in first scratch tensor (output chunk 0)
NGB = 16               # second tensor (groups 28..43)
NGC = NG - NGA - NGB   # third tensor (groups 44..48)
B = 8

_CACHE = {}
LAST_RESULTS = None  # test.py reads exec_time from here


def _build_program():
    nc = bacc.Bacc("TRN2", target_bir_lowering=False, debug=False,
                   enable_asserts=True)

    def din(name, shape, dt=F32):
        return nc.dram_tensor(name, shape, dt, kind="ExternalInput")

    x_c = din("x_c", [128, 3 * N], BF16)
    wq3 = din("wq3", [128, 3 * NH_KD], BF16)
    wk3 = din("wk3", [128, 3 * NH_KD], BF16)
    wv3 = din("wv3", [128, 3 * DH], BF16)
    wp8 = din("wp8", [128, 8 * DIM], BF16)
    vecs = din("vecs", [128, 113])
    w2bd = din("w2bd", [128, 128])
    ident = din("ident", [128, 128], BF16)
    identf8 = din("identf8", [128, 128], FP8)
    abt = din("abt", [NG * 128, N], FP8)

    out = nc.dram_tensor("out", [DIM, N], F32, kind="ExternalOutput")
    a2da = nc.dram_tensor("a2da", [NGA * 128, 896], BF16, kind="Internal")
    a2db = nc.dram_tensor("a2db", [NGB * 128, 896], BF16, kind="Internal")
    a2dc = nc.dram_tensor("a2dc", [NGC * 128, 896], BF16, kind="Internal")

    CH0 = slice(0, 392)          # logits free-dim chunks (within 2-bank psum)
    CH1 = slice(392, 784)
    PS0 = slice(0, 392)          # psum [128,1024]: bank0
    PS1 = slice(512, 904)        # bank1

    def psum2view(ps):
        # [128, 2, 392] view of a 2-bank psum tile's used region
        return ps[:].rearrange("p (a c) -> p a c", c=512)[:, :, 0:392]

    with tile.TileContext(nc) as tc:
        with (
            tc.tile_pool(name="consts", bufs=1) as consts,
            tc.tile_pool(name="persist", bufs=1) as persist,
        ):
            # ---- resident weights/constants -------------------------------
            def load_const(name, src_ap, shape, dt=F32):
                t = consts.tile(shape, dt, tag=name, name=name)
                nc.sync.dma_start(t[:], src_ap)
                return t

            wp_w = consts.tile([128, 8 * DIM], BF16, tag="wp_w", name="wp_w")
            wp_t = [wp_w[:, k * DIM:(k + 1) * DIM] for k in range(8)]
            vec_t = consts.tile([128, 113], F32, tag="vec_t", name="vec_t")
            bq_t = [vec_t[:, k:k + 1] for k in range(2)]
            bk_t = [vec_t[:, 2 + k:3 + k] for k in range(2)]
            bv_t = [vec_t[:, 4 + k:5 + k] for k in range(8)]
            bdw_t = [vec_t[:, 12 + k:13 + k] for k in range(8)]
            bp_t = [vec_t[:, 20 + k:21 + k] for k in range(3)]
            bth1_t = vec_t[:, 23:24]
            bth2_t = vec_t[:, 24:25]
            sq_t = [vec_t[:, 25 + k * 8:33 + k * 8] for k in range(2)]
            wtap_t = [vec_t[:, 41 + g * 9:50 + g * 9] for g in range(8)]
            w2bd_t = consts.tile([128, 128], F32, tag="w2bd", name="w2bd")
            ident_t = consts.tile([128, 128], BF16, tag="ident", name="ident")
            identf8_t = consts.tile([128, 128], FP8, tag="identf8",
                                    name="identf8")

            def load_small_consts():
                nc.sync.dma_start(vec_t[:], vecs.ap()[:])
                nc.sync.dma_start(w2bd_t[:], w2bd.ap()[:])
                nc.sync.dma_start(ident_t[:], ident.ap()[:])
                nc.sync.dma_start(identf8_t[:], identf8.ap()[:])

            # ---- persistent activations -----------------------------------
            vpad = [persist.tile([128, 900], BF16, tag=f"vpad{p}",
                                 name=f"vpad{p}") for p in range(8)]
            vt = [persist.tile([128, DH], BF16, tag=f"vt{m}", name=f"vt{m}")
                  for m in range(7)]
            osum = [persist.tile([128, N], BF16, tag=f"osum{p}",
                                 name=f"osum{p}") for p in range(8)]

            a2tap_cm = tc.tile_pool(name="a2tap", bufs=1)
            a2tap = a2tap_cm.__enter__()
            a2ta = [a2tap.tile([128, NGA * 128], BF16, tag=f"a2ta{m}",
                               name=f"a2ta{m}") for m in range(7)]
            a2tb = [a2tap.tile([128, (NGB + NGC) * 128], BF16, tag=f"a2tb{m}",
                               name=f"a2tb{m}") for m in range(7)]
            with tc.tile_pool(name="qk", bufs=1) as qkpool:
              # =========== Phase A: projections ============================
              with (
                tc.tile_pool(name="pa", bufs=3, space="PSUM") as pa,
                tc.tile_pool(name="ax", bufs=1) as axpool,
              ):
                x_w = axpool.tile([128, 3 * N], BF16, tag="xw", name="xw")
                for k in range(3):
                    nc.gpsimd.dma_start(x_w[:, k * N:(k + 1) * N],
                                        x_c.ap()[:, k * N:(k + 1) * N])
                x_t = [x_w[:, k * N:(k + 1) * N] for k in range(3)]

                def load_a(name, src_ap, shape, dt=F32):
                    t = axpool.tile(shape, dt, tag=name, name=name)
                    nc.sync.dma_start(t[:], src_ap)
                    return t

                wq_w = load_a("wq_w", wq3.ap()[:], [128, 3 * NH_KD], BF16)
                wk_w = load_a("wk_w", wk3.ap()[:], [128, 3 * NH_KD], BF16)
                wv_w = load_a("wv_w", wv3.ap()[:], [128, 3 * DH], BF16)
                wq_t = [wq_w[:, k * NH_KD:(k + 1) * NH_KD] for k in range(3)]
                wk_t = [wk_w[:, k * NH_KD:(k + 1) * NH_KD] for k in range(3)]
                wv_t = [wv_w[:, k * DH:(k + 1) * DH] for k in range(3)]
                load_small_consts()

                q_t = [axpool.tile([128, N], BF16, tag=f"q{k}", name=f"q{k}")
                       for k in range(2)]
                k_t = [qkpool.tile([128, N], BF16, tag=f"k{k}", name=f"kp{k}")
                       for k in range(2)]
                q2_t = [qkpool.tile([128, NG * 128], BF16, tag=f"q2{k}",
                                    name=f"q2{k}") for k in range(2)]

                # q and k projections: out[ot*128.., n]
                for (wts, bias, dst) in ((wq_t, bq_t, q_t), (wk_t, bk_t, k_t)):
                    for ot in range(2):
                        ps = pa.tile([128, 1024], F32, tag="pa", name="pa")
                        for ci, chs in enumerate((CH0, CH1)):
                            pchunk = ps[:, PS0] if ci == 0 else ps[:, PS1]
                            for kt in range(3):
                                nc.tensor.matmul(
                                    pchunk,
                                    lhsT=wts[kt][:, ot * 128:(ot + 1) * 128],
                                    rhs=x_t[kt][:, chs],
                                    start=(kt == 0), stop=(kt == 2))
                        nc.vector.tensor_scalar_add(dst[ot][:], psum2view(ps),
                                                    bias[ot])

                # Q2: 8 per-head-scaled copies of q, bf16
                for kt in range(2):
                    qv = q_t[kt][:].rearrange("p (a i) -> p a i", i=16)
                    q2v = q2_t[kt][:].rearrange("p (a g i) -> p a g i",
                                                g=8, i=16)
                    for (a0, a1) in ((0, 16), (16, NG)):
                        for g in range(8):
                            nc.vector.tensor_scalar_mul(
                                q2v[:, a0:a1, g, :], qv[:, a0:a1, :],
                                sq_t[kt][:, g:g + 1])

                # v projection straight into the zero-padded 30x30 grid
                for p in range(8):
                    vvz = vpad[p][:].rearrange("p (r c) -> p r c", c=30)
                    nc.gpsimd.memset(vvz[:, 0, :], 0.0)
                    nc.gpsimd.memset(vvz[:, 29, :], 0.0)
                    nc.gpsimd.memset(vvz[:, 1:29, 0], 0.0)
                    nc.gpsimd.memset(vvz[:, 1:29, 29], 0.0)
                    ps = pa.tile([128, 1024], F32, tag="pa", name="pa")
                    for ci in range(2):
                        pchunk = ps[:, PS0] if ci == 0 else ps[:, PS1]
                        for kt in range(3):
                            nc.tensor.matmul(
                                pchunk,
                                lhsT=wv_t[kt][:, p * 128:(p + 1) * 128],
                                rhs=x_t[kt][:, CH0 if ci == 0 else CH1],
                                start=(kt == 0), stop=(kt == 2))
                    vview = vpad[p][:].rearrange("p (r c) -> p r c", c=30)
                    rows = vview[:, 1:29, 1:29].rearrange(
                        "p (a r) c -> p a r c", a=2)
                    pin = psum2view(ps).rearrange("p a (r c) -> p a r c", c=28)
                    nc.scalar.activation(rows, pin, AF.Identity, bias=bv_t[p])

                # V^T tiles [m,(g,d)] directly from the projection (+bias via
                # a K=1 ones-row matmul)
                nc.gpsimd.memset(vt[6][:], 0.0)
                for mt in range(7):
                    M = 128 if mt < 6 else 16
                    msl = slice(mt * 128, mt * 128 + M)
                    ps = pa.tile([128, 1024], F32, tag="pa", name="pa")
                    for ci in range(2):
                        pchunk = ps[0:M, ci * 512:(ci + 1) * 512]
                        csl = slice(ci * 512, (ci + 1) * 512)
                        for kt in range(3):
                            nc.tensor.matmul(pchunk,
                                             lhsT=x_t[kt][:, msl],
                                             rhs=wv_t[kt][:, csl],
                                             start=(kt == 0), stop=(kt == 2))
                        nc.scalar.copy(vt[mt][0:M, csl], pchunk)

              # =========== Phase C: attention per 16-query group ===========
              with (
                  tc.tile_pool(name="pc", bufs=2, space="PSUM") as pc,
                  tc.tile_pool(name="cw", bufs=2) as cw,
                  tc.tile_pool(name="cz", bufs=3) as cz,
              ):
                  abt_v = abt.ap().rearrange("(a p) c -> p a c", p=128)
                  abtiles = {}
                  pending = []
                  a2cur = [None]

                  def fetch_ab(k):
                      nab = min(4, NG - k * 4)
                      t = cw.tile([128, 4 * N], FP8, tag="ab", name="ab",
                                  bufs=5)
                      nc.scalar.dma_start(
                          t[:, 0:nab * N].rearrange("p (a c) -> p a c", c=N),
                          abt_v[:, k * 4:k * 4 + nab, :])
                      abtiles[k] = t

                  for _k in range(4):
                      fetch_ab(_k)

                  zpad = cw.tile([128, 28 * 112], BF16, tag="zpad",
                                 name="zpad", bufs=1)
                  nc.gpsimd.memset(zpad[:], 0.0)
                  zv = zpad[:].rearrange("p (a c) -> p a c", c=112)
                  for (tns, ngr) in ((a2da, NGA), (a2db, NGB), (a2dc, NGC)):
                      dvz = tns.ap().rearrange("(a p) c -> p a c", p=128)
                      nc.gpsimd.dma_start(dvz[:, :, 784:896],
                                          zv[:, 0:ngr, :])
                  for gi in range(NG):
                      gsl = slice(gi * 128, (gi + 1) * 128)
                      if gi % 4 == 2 and gi // 4 + 4 <= (NG - 1) // 4:
                          fetch_ab(gi // 4 + 4)
                      ab4 = abtiles[gi // 4]
                      ab = ab4[:, (gi % 4) * N:(gi % 4 + 1) * N]

                      lg = pc.tile([128, 1024], F32, tag="lg", name="lg",
                                   bufs=2)
                      for ci, chs in enumerate((CH0, CH1)):
                          pchunk = lg[:, PS0] if ci == 0 else lg[:, PS1]
                          for kt in range(2):
                              nc.tensor.matmul(pchunk,
                                               lhsT=q2_t[kt][:, gsl],
                                               rhs=k_t[kt][:, chs],
                                               start=(kt == 0), stop=False)
                          nc.tensor.matmul(pchunk, lhsT=identf8_t[:],
                                           rhs=ab[:, chs],
                                           start=False, stop=True)

                      e = cw.tile([128, N], BF16, tag="e", name="e",
                                  bufs=3)
                      z = cz.tile([128, 1], F32, tag="z", name="z")
                      nc.scalar.activation(e[:], psum2view(lg), AF.Exp,
                                           bias=bth1_t, accum_out=z[:])

                      r = cz.tile([128, 1], F32, tag="r", name="r")
                      nc.vector.reciprocal(r[:], z[:])
                      w2s = cz.tile([128, 128], BF16, tag="w2s", name="w2s")
                      nc.vector.tensor_scalar_mul(w2s[:], w2bd_t[:], r[:])

                      pending.append((gi, e, w2s))
                      if gi == NG - 1:
                          flush = pending
                          pending = []
                      elif len(pending) > 2:
                          flush = [pending.pop(0)]
                      else:
                          flush = []
                      for (fgi, fe, fw2s) in flush:
                          a2p = pc.tile([128, 1024], F32, tag="a2p",
                                        name="a2p", bufs=2)
                          for ci, chs in enumerate((CH0, CH1)):
                              pchunk = a2p[:, PS0] if ci == 0 else a2p[:, PS1]
                              nc.tensor.matmul(pchunk, lhsT=fw2s[:],
                                               rhs=fe[:, chs],
                                               start=True, stop=True)
                          if fgi % 4 == 0:
                              a2w = cw.tile([128, 4 * 896], BF16, tag="a2",
                                            name="a2")
                              a2wv = a2w[:].rearrange("p (a c) -> p a c",
                                                      c=896)
                              a2cur[0] = a2wv
                          j = fgi % 4
                          nc.vector.tensor_scalar_add(
                              a2cur[0][:, j, 0:784], psum2view(a2p), bth2_t)
                          if j == 3 or fgi == NG - 1:
                              gi0 = fgi - j
                              nab = j + 1
                              if gi0 < NGA:
                                  dv = a2da.ap().rearrange("(a p) c -> p a c",
                                                           p=128)
                                  dst = dv[:, gi0:gi0 + nab, :]
                              elif gi0 < NGA + NGB:
                                  dv = a2db.ap().rearrange("(a p) c -> p a c",
                                                           p=128)
                                  dst = dv[:, gi0 - NGA:gi0 - NGA + nab, :]
                              else:
                                  dv = a2dc.ap().rearrange("(a p) c -> p a c",
                                                           p=128)
                                  g0c = gi0 - NGA - NGB
                                  dst = dv[:, g0c:g0c + nab, :]
                              nc.gpsimd.dma_start(dst[:, :, 0:784],
                                                  a2cur[0][:, 0:nab, 0:784])
                          if fgi >= NGA - 1 and (fgi - (NGA - 1)) % 3 == 0:
                              mt = (fgi - (NGA - 1)) // 3
                              if mt < 7:
                                  nc.sync.dma_start_transpose(
                                      a2ta[mt][:],
                                      a2da.ap()[:, mt * 128:(mt + 1) * 128])
                          pass

            # ======= Phase D: attn@v + depthwise conv, fused projection ====
            # Two passes over output n-chunks: chunk0 (rows 0..15, 448 cols =
            # groups 0..27 = the a2da half, transposed during phase C) and
            # chunk1 (rows 16..27, 336 cols = groups 28..48 = a2db).
            with (
                tc.tile_pool(name="pd", bufs=2, space="PSUM") as pd,
                tc.tile_pool(name="dg", bufs=4) as dgp,
                tc.tile_pool(name="pe", bufs=1, space="PSUM") as pe,
                tc.tile_pool(name="ow", bufs=1) as ow,
            ):
                nc.sync.dma_start(wp_w[:], wp8.ap()[:])
                # b-half transposes not yet issued inside phase C (the loop
                # only reaches mtb = NG-1 - (NGA+NGB-1))
                for mtb in range(0, 7):
                    nc.sync.dma_start_transpose(
                        a2tb[mtb][:, 0:NGB * 128],
                        a2db.ap()[:, mtb * 128:(mtb + 1) * 128])
                for mt in range(7):
                    nc.sync.dma_start_transpose(
                        a2tb[mt][:, NGB * 128:(NGB + NGC) * 128],
                        a2dc.ap()[:, mt * 128:(mt + 1) * 128])

                ot = [ow.tile([128, N], F32, tag=f"ot{mt}", name=f"ot{mt}")
                      for mt in range(3)]
                DCH = ((0, 16, 28, 448), (16, 12, 21, 336))
                for ci, (r0, nr, ngr, w) in enumerate(DCH):
                    csl = slice(0, 448) if ci == 0 else slice(448, 784)
                    half = a2ta if ci == 0 else a2tb
                    pp = [pe.tile([128, w], F32, tag=f"pp{ci}{mt}",
                                  name=f"pp{ci}{mt}") for mt in range(3)]
                    prev_e = None
                    for g in range(8):
                        po = pd.tile([128, w], F32, tag="po", name="po")
                        dgt = [dgp.tile([128, 128], BF16, tag="dg", name="dg")
                               for _ in range(9)]
                        for t in range(9):
                            nc.vector.tensor_scalar_mul(
                                dgt[t][:], ident_t[:], wtap_t[g][:, t:t + 1])
                        vv = vpad[g][:].rearrange("p (r c) -> p r c", c=30)
                        for t in range(9):
                            dy, dx = t // 3, t % 3
                            srcv = vv[:, r0 + dy:r0 + dy + nr, dx:dx + 28]
                            nc.tensor.matmul(po[:], lhsT=dgt[t][:], rhs=srcv,
                                             start=(t == 0), stop=False)
                        for mt in range(7):
                            cols = half[mt][:].rearrange(
                                "p (a G i) -> p a G i", G=8, i=16)[:, :, g, :]
                            nc.tensor.matmul(
                                po[:],
                                lhsT=vt[mt][:, g * 128:(g + 1) * 128],
                                rhs=cols, start=False, stop=(mt == 6))
                        nc.scalar.activation(osum[g][:, csl], po[:],
                                             AF.Identity, bias=bdw_t[g])
                        if prev_e is not None:
                            for mt in range(3):
                                nc.tensor.matmul(
                                    pp[mt][:],
                                    lhsT=wp_t[prev_e][:,
                                                      mt * 128:(mt + 1) * 128],
                                    rhs=osum[prev_e][:, csl],
                                    start=(prev_e == 0), stop=False)
                        prev_e = g
                    for mt in range(3):
                        nc.tensor.matmul(
                            pp[mt][:],
                            lhsT=wp_t[7][:, mt * 128:(mt + 1) * 128],
                            rhs=osum[7][:, csl],
                            start=False, stop=True)
                    for mt in range(3):
                        nc.scalar.activation(ot[mt][:, csl], pp[mt][:],
                                             AF.Identity, bias=bp_t[mt])
                        nc.scalar.dma_start(
                            out.ap()[mt * 128:(mt + 1) * 128, csl],
                            ot[mt][:, csl])

            a2tap_cm.__exit__(None, None, None)

    nc.compile()
    return nc


def _prep_common(inputs):
    f32 = np.float32
    scale = np.float32(KD ** -0.5)
    q_s, q_b = inputs["q_s"], inputs["q_b"]
    k_s, k_b = inputs["k_s"], inputs["k_b"]
    v_s, v_b = inputs["v_s"], inputs["v_b"]
    p_s, p_b = inputs["p_s"], inputs["p_b"]

    Wq = np.asarray(inputs["Wq"], f32) * np.asarray(q_s, f32)[:, None] * scale
    bqv = (np.asarray(q_s, f32) * np.asarray(inputs["bq"], f32)
           + np.asarray(q_b, f32)) * scale
    Wk = np.asarray(inputs["Wk"], f32) * np.asarray(k_s, f32)[:, None]
    bkv = np.asarray(k_s, f32) * np.asarray(inputs["bk"], f32) + np.asarray(k_b, f32)
    Wv = np.asarray(inputs["Wv"], f32) * np.asarray(v_s, f32)[:, None]
    bvv = np.asarray(v_s, f32) * np.asarray(inputs["bv"], f32) + np.asarray(v_b, f32)
    Wp = np.asarray(inputs["Wp"], f32) * np.asarray(p_s, f32)[:, None]
    bpv = np.asarray(p_s, f32) * np.asarray(inputs["bp"], f32) + np.asarray(p_b, f32)

    Wth1 = np.asarray(inputs["Wth1"], f32)
    bth1 = np.asarray(inputs["bth1"], f32)
    Wth2 = np.asarray(inputs["Wth2"], f32)
    bth2 = np.asarray(inputs["bth2"], f32)

    # talking-head-1 folded bias table, rows ordered (group, g, i); bth1 is
    # applied separately as the ACT exp bias
    ab1 = Wth1 @ np.asarray(inputs["attention_biases"], f32)      # [8, 784]
    idx = np.asarray(inputs["bias_idxs"])                          # [784, 784]
    ab_full = ab1[:, idx]                                          # [8,784,784]
    abt = np.ascontiguousarray(
        ab_full.reshape(8, NG, 16, N).transpose(1, 0, 2, 3)
    ).reshape(NG * 128, N).astype(F8)

    # depthwise weights folded with BN
    wvl = np.asarray(inputs["Wvl"], f32)[:, 0, :, :].reshape(DH, 9)
    vl_s = np.asarray(inputs["vl_s"], f32)
    wtap = wvl * vl_s[:, None]
    bdw = (np.asarray(inputs["bvl"], f32) * vl_s
           + np.asarray(inputs["vl_b"], f32))

    def ktile_pack(wT, nk):
        # [nk*128, C] -> [128, nk*C] with k-tile-major free dim
        C = wT.shape[1]
        return np.ascontiguousarray(
            wT.reshape(nk, 128, C).transpose(1, 0, 2).reshape(128, nk * C))

    sqv = np.repeat(Wth1.T, KD, axis=0).astype(f32)                # [256, 8]
    vecs = np.zeros((128, 113), f32)
    vecs[:, 0:2] = bqv.reshape(2, 128).T
    vecs[:, 2:4] = bkv.reshape(2, 128).T
    vecs[:, 4:12] = bvv.reshape(8, 128).T
    s2 = Wth2.sum(axis=1) + N * bth2                   # [8] per out-head
    bdw2 = bdw + bvv * np.repeat(s2, D)
    vecs[:, 12:20] = bdw2.reshape(8, 128).T
    vecs[:, 20:23] = bpv.reshape(3, 128).T
    vecs[:, 23] = np.repeat(bth1, 16)
    vecs[:, 24] = np.repeat(bth2, 16)
    vecs[:, 25:33] = sqv[0:128]
    vecs[:, 33:41] = sqv[128:256]
    for g in range(8):
        vecs[:, 41 + g * 9:50 + g * 9] = wtap[g * 128:(g + 1) * 128]

    common = {
        "wq3": ktile_pack(np.ascontiguousarray(Wq.T), 3).astype(BF),
        "wk3": ktile_pack(np.ascontiguousarray(Wk.T), 3).astype(BF),
        "wv3": ktile_pack(np.ascontiguousarray(Wv.T), 3).astype(BF),
        "wp8": ktile_pack(np.ascontiguousarray(Wp.T), 8).astype(BF),
        "vecs": vecs,
        "w2bd": np.kron(Wth2.T, np.eye(16, dtype=f32)).astype(f32),
        "ident": np.eye(128, dtype=f32).astype(BF),
        "identf8": np.eye(128, dtype=f32).astype(F8),
        "abt": abt,
    }
    return common


def kernel(**inputs):
    global LAST_RESULTS
    if "nc" not in _CACHE:
        _CACHE["nc"] = _build_program()
    nc = _CACHE["nc"]

    common = _prep_common(inputs)
    x = np.asarray(inputs["x"], np.float32)          # [8, 384, 28, 28]
    in_maps = []
    for c in range(B):
        m = dict(common)
        xc = x[c].reshape(3, 128, N).transpose(1, 0, 2).reshape(128, 3 * N)
        m["x_c"] = np.ascontiguousarray(xc).astype(BF)
        in_maps.append(m)

    import os
    trace = bool(int(os.environ.get("KERNEL_TRACE", "0")))
    res = run_bass_kernel_spmd(nc, in_maps, core_ids=list(range(B)),
                               trace=trace)
    LAST_RESULTS = res
    out = np.stack([res.results[c]["out"].reshape(DIM, RES, RES)
                    for c in range(B)])
    return out.astype(np.float32)



# revision 2
# speedup vs baseline: 1.0095x; 1.0095x over previous
"""Trainium2 Bass kernel for nn_Attention4D (EfficientViT-style attention).

Strategy (8 NeuronCores, data-parallel over batch B=8, one element per core):
  - BN folded into conv weights on host; talking-head-1 folded into
    per-head-scaled queries (Q2) so q@k contracts 256 channels.
  - Q2/K/e/w2s and the depthwise taps are fp8e4m3; the big matmuls use
    MatmulPerfMode.DoubleRow (2 K-tiles per instruction at 0.5 cycles/row):
      * logits: one DoubleRow (K=256) per 392-col chunk
      * rel-pos bias: DoubleRow with [I|0] weights
      * talking-head-2: produced TRANSPOSED as a2T = e_chunk^T @ w2s per
        (group, key-chunk) - keys land on partitions, so the DRAM transpose
        round-trip of the previous design disappears entirely.  Softmax
        normalization (1/z) rides inside w2s; bth2 is applied in phase D as
        a per-channel bias built from accum-reduced v sums.
      * depthwise 3x3: taps paired 2-per-DoubleRow with host-packed fp8
        diagonal weight pairs.
  - a2T / V stay bf16 (fp8 there fails the 2e-2 tolerance).
  - Phase C is software-pipelined; v/vt projections and phase-D head blocks
    are interleaved into the group loop as PE filler, with psum->sbuf copy
    work placed on ACT/DVE by a least-loaded heuristic.
"""

import sys

sys.path.insert(0, "/opt/trn_rl_repo")

import numpy as np
import ml_dtypes

import concourse.bass as bass
import concourse.tile as tile
from concourse import bacc, mybir
from concourse.bass_utils import run_bass_kernel_spmd
from bass_rust import AP

F32 = mybir.dt.float32
BF16 = mybir.dt.bfloat16
FP8 = mybir.dt.float8e4
AF = mybir.ActivationFunctionType
ALU = mybir.AluOpType
DR = mybir.MatmulPerfMode.DoubleRow
BF = ml_dtypes.bfloat16
F8 = ml_dtypes.float8_e4m3

HEADS, KD, AR, RES, DIM = 8, 32, 4, 28, 384
D = AR * KD            # 128
DH = HEADS * D         # 1024
NH_KD = HEADS * KD     # 256
N = RES * RES          # 784
NG = N // 16           # 49 groups of 16 queries
B = 8

_CACHE = {}
LAST_RESULTS = None  # test.py reads exec_time from here


def _sv(base, off_elems, dims):
    """Custom strided view: keep base AP's partition dim, free dims given as
    [(stride, size), ...]."""
    ap = [list(base.ap[0])] + [[s, n] for (s, n) in dims]
    return AP(base.tensor, base.offset + off_elems, ap)


def _build_program():
    nc = bacc.Bacc("TRN2", target_bir_lowering=False, debug=False,
                   enable_asserts=True)

    def din(name, shape, dt=F32):
        return nc.dram_tensor(name, shape, dt, kind="ExternalInput")

    x_c = din("x_c", [128, 3 * N], BF16)
    wq3 = din("wq3", [128, 3 * NH_KD], BF16)
    wk3 = din("wk3", [128, 3 * NH_KD], BF16)
    wv3 = din("wv3", [128, 3 * DH], BF16)
    wp8 = din("wp8", [128, 8 * DIM], BF16)
    vecs = din("vecs", [128, 48])
    w2bd = din("w2bd", [128, 128])
    idz = din("idz", [128, 256], FP8)
    dwp = din("dwp", [128, 8 * 9 * 128], FP8)
    abt = din("abt", [128, NG * N], FP8)

    out = nc.dram_tensor("out", [DIM, N], F32, kind="ExternalOutput")

    CH = (slice(0, 392), slice(392, 784))
    PSC = (slice(0, 392), slice(512, 904))

    def psum2view(ps):
        return ps[:].rearrange("p (a c) -> p a c", c=512)[:, :, 0:392]

    # engine-balance bookkeeping for flexible ACT/DVE ops
    ebusy = {"act": 0.0, "dve": 0.0}

    def flex(free, act_fn, dve_fn, accum=False):
        ca = (free + 250) * 0.833 + (187 if accum else 0)
        cd = (free + 150) * 1.042
        if ebusy["act"] + ca < ebusy["dve"] + cd:
            ebusy["act"] += ca
            act_fn()
        else:
            ebusy["dve"] += cd
            dve_fn()

    def act_only(free, fn, accum=False):
        ebusy["act"] += (free + 250) * 0.833 + (187 if accum else 0)
        fn()

    def dve_only(free, fn):
        ebusy["dve"] += (free + 150) * 1.042
        fn()

    with tile.TileContext(nc) as tc:
        with (
            tc.tile_pool(name="consts", bufs=1) as consts,
            tc.tile_pool(name="persist", bufs=1) as persist,
        ):
            vec_t = consts.tile([128, 48], F32, tag="vec_t", name="vec_t")
            bq_t = [vec_t[:, k:k + 1] for k in range(2)]
            bk_t = [vec_t[:, 2 + k:3 + k] for k in range(2)]
            bv_t = [vec_t[:, 4 + k:5 + k] for k in range(8)]
            bdw2_t = [vec_t[:, 12 + k:13 + k] for k in range(8)]
            bp_t = [vec_t[:, 20 + k:21 + k] for k in range(3)]
            bth1_t = vec_t[:, 23:24]
            bth2g_t = [vec_t[:, 24 + k:25 + k] for k in range(8)]
            sq_t = [[vec_t[:, 32 + kt * 8 + g:33 + kt * 8 + g]
                     for g in range(8)] for kt in range(2)]
            w2bd_t = consts.tile([128, 128], F32, tag="w2bd", name="w2bd")
            idz_t = consts.tile([128, 256], FP8, tag="idz", name="idz")
            dwp_t = consts.tile([128, 8 * 9 * 128], FP8, tag="dwp",
                                name="dwp")
            wp_w = consts.tile([128, 8 * DIM], BF16, tag="wp_w", name="wp_w")
            wp_t = [wp_w[:, k * DIM:(k + 1) * DIM] for k in range(8)]

            # persistent activations
            q2 = persist.tile([128, 2 * NG * 128], FP8, tag="q2", name="q2")
            q2tv = q2[:].rearrange("p (t c) -> p t c", c=NG * 128)
            k2 = persist.tile([128, 2 * N], FP8, tag="k2", name="k2")
            k2v = k2[:].rearrange("p (t c) -> p t c", c=N)
            vpad = [persist.tile([128, 900], FP8, tag=f"vpad{p}",
                                 name=f"vpad{p}") for p in range(8)]
            vt = [persist.tile([128, DH], BF16, tag=f"vt{m}", name=f"vt{m}")
                  for m in range(7)]
            a2t = persist.tile([128, 7 * NG * 128], BF16, tag="a2t",
                               name="a2t")
            a2tv = a2t[:].rearrange("p (m a c) -> p m a c", a=NG, c=128)
            osum = [persist.tile([128, N], BF16, tag=f"osum{p}",
                                 name=f"osum{p}") for p in range(8)]
            vsum = persist.tile([128, 8], F32, tag="vsum", name="vsum")
            bias2 = persist.tile([128, 8], F32, tag="bias2", name="bias2")
            qb = [persist.tile([128, N], BF16, tag=f"qb{k}", name=f"qb{k}")
                  for k in range(2)]

            with (
                tc.tile_pool(name="ax", bufs=1) as axpool,
                tc.tile_pool(name="pbig", bufs=2, space="PSUM") as pbig,
                tc.tile_pool(name="pa2", bufs=2, space="PSUM") as pa2,
                tc.tile_pool(name="cw", bufs=1) as cw,
                tc.tile_pool(name="cz", bufs=3) as cz,
            ):
                # ---------------- loads ---------------------------------
                x_w = axpool.tile([128, 3 * N], BF16, tag="xw", name="xw")
                for k in range(3):
                    nc.gpsimd.dma_start(x_w[:, k * N:(k + 1) * N],
                                        x_c.ap()[:, k * N:(k + 1) * N])
                x_t = [x_w[:, k * N:(k + 1) * N] for k in range(3)]

                def load_a(name, src, shape, dt):
                    t = axpool.tile(shape, dt, tag=name, name=name)
                    nc.sync.dma_start(t[:], src)
                    return t

                wq_w = load_a("wq_w", wq3.ap()[:], [128, 3 * NH_KD], BF16)
                wk_w = load_a("wk_w", wk3.ap()[:], [128, 3 * NH_KD], BF16)
                wv_w = load_a("wv_w", wv3.ap()[:], [128, 3 * DH], BF16)
                wq_t = [wq_w[:, k * NH_KD:(k + 1) * NH_KD] for k in range(3)]
                wk_t = [wk_w[:, k * NH_KD:(k + 1) * NH_KD] for k in range(3)]
                wv_t = [wv_w[:, k * DH:(k + 1) * DH] for k in range(3)]
                nc.sync.dma_start(vec_t[:], vecs.ap()[:])
                nc.sync.dma_start(w2bd_t[:], w2bd.ap()[:])
                nc.sync.dma_start(idz_t[:], idz.ap()[:])
                nc.sync.dma_start(dwp_t[:], dwp.ap()[:])

                # ---------------- q/k projections ------------------------
                for (wts, bias, fp8out) in ((wq_t, bq_t, False),
                                            (wk_t, bk_t, True)):
                    for ot in range(2):
                        ps = pbig.tile([128, 1024], F32, tag="big",
                                       name="big")
                        for ci in range(2):
                            pchunk = ps[:, PSC[ci]]
                            for kt in range(3):
                                nc.tensor.matmul(
                                    pchunk,
                                    lhsT=wts[kt][:, ot * 128:(ot + 1) * 128],
                                    rhs=x_t[kt][:, CH[ci]],
                                    start=(kt == 0), stop=(kt == 2))
                        if fp8out:
                            dst = k2v[:, ot, :].rearrange(
                                "p (a c) -> p a c", c=392)
                        else:
                            dst = qb[ot][:].rearrange(
                                "p (a c) -> p a c", c=392)
                        dve_only(784, lambda d=dst, p=ps, b=bias[ot]:
                                 nc.vector.tensor_scalar_add(
                                     d, psum2view(p), b))

                # q2 fp8 prep, batched by group ranges; batch 0 up front
                q2_batches = [(0, 8), (8, 21), (21, 34), (34, 49)]
                q2_ops = []
                for (a0, a1) in q2_batches[1:]:
                    for kt in range(2):
                        for g in range(8):
                            q2_ops.append((kt, g, a0, a1))

                def emit_q2(kt, g, a0, a1):
                    dst = _sv(q2[:], kt * NG * 128 + a0 * 128 + g * 16,
                              [(128, a1 - a0), (1, 16)])
                    src = _sv(qb[kt][:], a0 * 16, [(16, a1 - a0), (1, 16)])
                    nc.gpsimd.tensor_scalar_mul(dst, src, sq_t[kt][g])

                for kt in range(2):
                    for g in range(8):
                        emit_q2(kt, g, 0, 8)

                # ---------------- filler task queues ---------------------
                def v_task(p):
                    def go():
                        vvz = vpad[p][:].rearrange("p (r c) -> p r c", c=30)
                        nc.gpsimd.memset(vvz[:, 0, :], 0.0)
                        nc.gpsimd.memset(vvz[:, 29, :], 0.0)
                        nc.gpsimd.memset(vvz[:, 1:29, 0], 0.0)
                        nc.gpsimd.memset(vvz[:, 1:29, 29], 0.0)
                        ps = pbig.tile([128, 1024], F32, tag="big",
                                       name="big")
                        for ci in range(2):
                            for kt in range(3):
                                nc.tensor.matmul(
                                    ps[:, PSC[ci]],
                                    lhsT=wv_t[kt][:, p * 128:(p + 1) * 128],
                                    rhs=x_t[kt][:, CH[ci]],
                                    start=(kt == 0), stop=(kt == 2))
                        rows = vvz[:, 1:29, 1:29].rearrange(
                            "p (a r) c -> p a r c", a=2)
                        pin = psum2view(ps).rearrange(
                            "p a (r c) -> p a r c", c=28)
                        vs = vsum[:, p:p + 1]
                        flex(784,
                             lambda: nc.scalar.activation(
                                 rows, pin, AF.Identity, bias=bv_t[p],
                                 accum_out=vs),
                             lambda: nc.vector.tensor_scalar(
                                 rows, pin, bv_t[p], 0.0, ALU.add,
                                 ALU.add, accum_out=vs),
                             accum=True)
                        if p == 7:
                            for g in range(8):
                                nc.vector.scalar_tensor_tensor(
                                    bias2[:, g:g + 1], vsum[:, g:g + 1],
                                    bth2g_t[g], bdw2_t[g],
                                    ALU.mult, ALU.add)
                    return go

                def vt_task(mt):
                    def go():
                        M = 128 if mt < 6 else 16
                        if mt == 6:
                            nc.gpsimd.memset(vt[6][:], 0.0)
                        msl = slice(mt * 128, mt * 128 + M)
                        ps = pbig.tile([128, 1024], F32, tag="big",
                                       name="big")
                        for ci in range(2):
                            pchunk = ps[0:M, ci * 512:(ci + 1) * 512]
                            wsl = slice(ci * 512, (ci + 1) * 512)
                            for kt in range(3):
                                nc.tensor.matmul(pchunk,
                                                 lhsT=x_t[kt][:, msl],
                                                 rhs=wv_t[kt][:, wsl],
                                                 start=(kt == 0),
                                                 stop=(kt == 2))
                        flex(1024,
                             lambda p=ps, m=M:
                             nc.scalar.copy(vt[mt][0:m, :], p[0:m, :]),
                             lambda p=ps, m=M:
                             nc.vector.tensor_scalar_add(
                                 vt[mt][0:m, :], p[0:m, :], 0.0))
                    return go

                DCH = ((0, 16, 0, 28, 448), (16, 12, 28, NG, 336))

                def dw_attnv(g, ci, pool, tag, tilew=1024):
                    r0, nr, a0, a1, w = DCH[ci]
                    csl = slice(r0 * 28, r0 * 28 + w)
                    po = pool.tile([128, tilew], F32, tag=tag, name=tag)
                    pov = po[:, 0:w]
                    vflat = vpad[g][:]
                    for p in range(4):
                        t0 = 2 * p
                        dy0, dx0 = t0 // 3, t0 % 3
                        dy1, dx1 = (t0 + 1) // 3, (t0 + 1) % 3
                        delta = (dy1 - dy0) * 30 + (dx1 - dx0)
                        rhs = _sv(vflat, (r0 + dy0) * 30 + dx0,
                                  [(delta, 2), (30, nr), (1, 28)])
                        lw = dwp_t[:, g * 1152 + p * 256:
                                   g * 1152 + p * 256 + 256].rearrange(
                                       "p (t c) -> p t c", c=128)
                        nc.tensor.matmul(pov, lhsT=lw, rhs=rhs,
                                         start=(p == 0), stop=False,
                                         perf_mode=DR)
                    rhs8 = _sv(vflat, (r0 + 2) * 30 + 2,
                               [(30, nr), (1, 28)])
                    nc.tensor.matmul(
                        pov,
                        lhsT=dwp_t[:, g * 1152 + 1024:g * 1152 + 1152],
                        rhs=rhs8, start=False, stop=False)
                    for mt in range(7):
                        cols = a2tv[:, mt, a0:a1, g * 16:(g + 1) * 16]
                        nc.tensor.matmul(
                            pov, lhsT=vt[mt][:, g * 128:(g + 1) * 128],
                            rhs=cols, start=False, stop=(mt == 6))
                    flex(w,
                         lambda: nc.scalar.activation(
                             osum[g][:, csl], pov, AF.Identity,
                             bias=bias2[:, g:g + 1]),
                         lambda: nc.vector.tensor_scalar_add(
                             osum[g][:, csl], pov, bias2[:, g:g + 1]))

                tasks_early = [v_task(p) for p in range(8)] + \
                              [vt_task(mt) for mt in range(7)]
                tasks_late = [(lambda gg=g: dw_attnv(gg, 0, pa2, "a2ps"))
                              for g in range(8)]

                # ---------------- phase C group loop ---------------------
                abtiles = {}

                def fetch_ab(kb):
                    nab = min(4, NG - kb * 4)
                    t = cw.tile([128, 4 * N + 392], FP8, tag="ab", name="ab",
                                bufs=5)
                    if kb < 5:
                        nc.gpsimd.memset(t[:, 4 * N:], 0.0)
                    nc.gpsimd.dma_start(
                        t[:, 0:nab * N],
                        abt.ap()[:, kb * 4 * N:(kb * 4 + nab) * N])
                    abtiles[kb] = t

                for _k in range(4):
                    fetch_ab(_k)

                pending = []

                def th2t_and_copy(gi, e_t, w2s_t):
                    a2ps = pa2.tile([128, 1024], F32, tag="a2ps",
                                    name="a2ps")
                    e2v = e_t[:].rearrange("p (t c) -> p t c", c=896)
                    w2sv = w2s_t[:].rearrange("p (t c) -> p t c", c=128)
                    for m in range(7):
                        nc.tensor.matmul(
                            a2ps[:, m * 128:(m + 1) * 128],
                            lhsT=e2v[:, :, m * 128:(m + 1) * 128],
                            rhs=w2sv,
                            start=True, stop=True, perf_mode=DR)
                    src = a2ps[:, 0:896].rearrange("p (m c) -> p m c",
                                                   c=128)
                    dst = a2tv[:, :, gi, :]
                    flex(896,
                         lambda: nc.scalar.copy(dst, src),
                         lambda: nc.vector.tensor_scalar_add(dst, src, 0.0))

                for gi in range(NG):
                    if gi % 4 == 2 and gi // 4 + 4 <= (NG - 1) // 4:
                        fetch_ab(gi // 4 + 4)
                    ab4 = abtiles[gi // 4]
                    j = gi % 4

                    lg = pbig.tile([128, 1024], F32, tag="big", name="big")
                    for ci in range(2):
                        pchunk = lg[:, PSC[ci]]
                        nc.tensor.matmul(
                            pchunk, lhsT=q2tv[:, :, gi * 128:(gi + 1) * 128],
                            rhs=k2v[:, :, CH[ci]],
                            start=True, stop=False, perf_mode=DR)
                        abrhs = _sv(ab4[:], j * N + ci * 392,
                                    [(392, 2), (1, 392)])
                        nc.tensor.matmul(
                            pchunk,
                            lhsT=idz_t[:].rearrange("p (t c) -> p t c",
                                                    c=128),
                            rhs=abrhs,
                            start=False, stop=True, perf_mode=DR)

                    e_t = cw.tile([128, 1792], FP8, tag="e", name="e",
                                  bufs=3)
                    if gi < 3:
                        nc.gpsimd.memset(e_t[:, 784:1792], 0.0)
                    z = cz.tile([128, 1], F32, tag="z", name="z")
                    ev = e_t[:, 0:784].rearrange("p (a c) -> p a c", c=392)
                    act_only(784, lambda: nc.scalar.activation(
                        ev, psum2view(lg), AF.Exp, bias=bth1_t,
                        accum_out=z[:]), accum=True)

                    w2s_t = cw.tile([128, 256], FP8, tag="w2s", name="w2s",
                                    bufs=3)
                    if gi < 3:
                        nc.gpsimd.memset(w2s_t[:, 128:256], 0.0)
                    r = cz.tile([128, 1], F32, tag="r", name="r")
                    nc.vector.reciprocal(r[:], z[:])
                    dve_only(128, lambda: nc.vector.tensor_scalar_mul(
                        w2s_t[:, 0:128], w2bd_t[:], r[:]))

                    pending.append((gi, e_t, w2s_t))
                    if len(pending) > 2:
                        th2t_and_copy(*pending.pop(0))

                    # fillers
                    if gi < 28:
                        if tasks_early and (gi * 15) // 28 > \
                                14 - len(tasks_early):
                            tasks_early.pop(0)()
                    elif gi >= 30 and tasks_late and (gi - 30) % 2 == 0:
                        tasks_late.pop(0)()
                    for _ in range(2):
                        if q2_ops:
                            emit_q2(*q2_ops.pop(0))

                while pending:
                    th2t_and_copy(*pending.pop(0))
                while tasks_early:
                    tasks_early.pop(0)()
                while tasks_late:
                    tasks_late.pop(0)()

            # ================= phase D tail ==========================
            with (
                tc.tile_pool(name="pd", bufs=2, space="PSUM") as pd,
                tc.tile_pool(name="pe", bufs=1, space="PSUM") as pe,
                tc.tile_pool(name="ow", bufs=1) as ow,
            ):
                nc.sync.dma_start(wp_w[:], wp8.ap()[:])
                ot = [ow.tile([128, N], F32, tag=f"ot{mt}", name=f"ot{mt}")
                      for mt in range(3)]

                # D1 head blocks interleaved with chunk-0 p-projection
                pp0 = [pe.tile([128, 448], F32, tag=f"pp0{mt}",
                               name=f"pp0{mt}") for mt in range(3)]
                for g in range(8):
                    dw_attnv(g, 1, pd, "po2", tilew=448)
                    for mt in range(3):
                        nc.tensor.matmul(
                            pp0[mt][:],
                            lhsT=wp_t[g][:, mt * 128:(mt + 1) * 128],
                            rhs=osum[g][:, 0:448],
                            start=(g == 0), stop=(g == 7))
                for mt in range(3):
                    flex(448, lambda m=mt: nc.scalar.activation(
                        ot[m][:, 0:448], pp0[m][:], AF.Identity,
                        bias=bp_t[m]),
                        lambda m=mt: nc.vector.tensor_scalar_add(
                            ot[m][:, 0:448], pp0[m][:], bp_t[m]))
                    nc.gpsimd.dma_start(out.ap()[mt * 128:(mt + 1) * 128,
                                                 0:448], ot[mt][:, 0:448])

                pp1 = [pe.tile([128, 336], F32, tag=f"pp1{mt}",
                               name=f"pp1{mt}") for mt in range(3)]
                for g in range(8):
                    for mt in range(3):
                        nc.tensor.matmul(
                            pp1[mt][:],
                            lhsT=wp_t[g][:, mt * 128:(mt + 1) * 128],
                            rhs=osum[g][:, 448:784],
                            start=(g == 0), stop=(g == 7))
                for mt in range(3):
                    flex(336, lambda m=mt: nc.scalar.activation(
                        ot[m][:, 448:784], pp1[m][:], AF.Identity,
                        bias=bp_t[m]),
                        lambda m=mt: nc.vector.tensor_scalar_add(
                            ot[m][:, 448:784], pp1[m][:], bp_t[m]))
                    nc.gpsimd.dma_start(out.ap()[mt * 128:(mt + 1) * 128,
                                                 448:784],
                                        ot[mt][:, 448:784])

    nc.compile()
    return nc


def _prep_common(inputs):
    f32 = np.float32
    scale = np.float32(KD ** -0.5)
    q_s, q_b = inputs["q_s"], inputs["q_b"]
    k_s, k_b = inputs["k_s"], inputs["k_b"]
    v_s, v_b = inputs["v_s"], inputs["v_b"]
    p_s, p_b = inputs["p_s"], inputs["p_b"]

    Wq = np.asarray(inputs["Wq"], f32) * np.asarray(q_s, f32)[:, None] * scale
    bqv = (np.asarray(q_s, f32) * np.asarray(inputs["bq"], f32)
           + np.asarray(q_b, f32)) * scale
    Wk = np.asarray(inputs["Wk"], f32) * np.asarray(k_s, f32)[:, None]
    bkv = np.asarray(k_s, f32) * np.asarray(inputs["bk"], f32) \
        + np.asarray(k_b, f32)
    Wv = np.asarray(inputs["Wv"], f32) * np.asarray(v_s, f32)[:, None]
    bvv = np.asarray(v_s, f32) * np.asarray(inputs["bv"], f32) \
        + np.asarray(v_b, f32)
    Wp = np.asarray(inputs["Wp"], f32) * np.asarray(p_s, f32)[:, None]
    bpv = np.asarray(p_s, f32) * np.asarray(inputs["bp"], f32) \
        + np.asarray(p_b, f32)

    Wth1 = np.asarray(inputs["Wth1"], f32)
    bth1 = np.asarray(inputs["bth1"], f32)
    Wth2 = np.asarray(inputs["Wth2"], f32)
    bth2 = np.asarray(inputs["bth2"], f32)

    # th1-folded rel-pos bias table, rows (group, g, i)
    ab1 = Wth1 @ np.asarray(inputs["attention_biases"], f32)
    idx = np.asarray(inputs["bias_idxs"])
    ab_full = ab1[:, idx]                                  # [8,784,784]
    # rows (group, g, i) -> partition-major [128, NG*784] so each 4-group
    # fetch is one contiguous segment per partition
    abt = np.ascontiguousarray(
        ab_full.reshape(8, NG, 16, N).transpose(1, 0, 2, 3)
    ).reshape(NG, 128, N).transpose(1, 0, 2).reshape(128, NG * N)
    abt = np.ascontiguousarray(abt).astype(F8)

    # depthwise weights folded with BN -> fp8 diagonal pair blocks
    wvl = np.asarray(inputs["Wvl"], f32)[:, 0, :, :].reshape(DH, 9)
    vl_s = np.asarray(inputs["vl_s"], f32)
    wtap = wvl * vl_s[:, None]                             # [1024, 9]
    bdw = (np.asarray(inputs["bvl"], f32) * vl_s
           + np.asarray(inputs["vl_b"], f32))
    dwp = np.zeros((128, 8 * 9 * 128), f32)
    for g in range(8):
        for t in range(9):
            blk = np.zeros((128, 128), f32)
            np.fill_diagonal(blk, wtap[g * 128:(g + 1) * 128, t])
            dwp[:, g * 1152 + t * 128:g * 1152 + (t + 1) * 128] = blk

    def ktile_pack(wT, nk):
        C = wT.shape[1]
        return np.ascontiguousarray(
            wT.reshape(nk, 128, C).transpose(1, 0, 2).reshape(128, nk * C))

    sqv = np.repeat(Wth1.T, KD, axis=0).astype(f32)        # [256, 8]
    vecs = np.zeros((128, 48), f32)
    vecs[:, 0:2] = bqv.reshape(2, 128).T
    vecs[:, 2:4] = bkv.reshape(2, 128).T
    vecs[:, 4:12] = bvv.reshape(8, 128).T
    s2p = Wth2.sum(axis=1)                                 # no N*bth2 term
    bdw2 = bdw + bvv * np.repeat(s2p, D)
    vecs[:, 12:20] = bdw2.reshape(8, 128).T
    vecs[:, 20:23] = bpv.reshape(3, 128).T
    vecs[:, 23] = np.repeat(bth1, 16)
    for g in range(8):
        vecs[:, 24 + g] = bth2[g]
    vecs[:, 32:40] = sqv[0:128]
    vecs[:, 40:48] = sqv[128:256]

    common = {
        "wq3": ktile_pack(np.ascontiguousarray(Wq.T), 3).astype(BF),
        "wk3": ktile_pack(np.ascontiguousarray(Wk.T), 3).astype(BF),
        "wv3": ktile_pack(np.ascontiguousarray(Wv.T), 3).astype(BF),
        "wp8": ktile_pack(np.ascontiguousarray(Wp.T), 8).astype(BF),
        "vecs": vecs,
        "w2bd": np.kron(Wth2.T, np.eye(16, dtype=f32)).astype(f32),
        "idz": np.concatenate([np.eye(128, dtype=f32),
                               np.zeros((128, 128), f32)],
                              axis=1).astype(F8),
        "dwp": dwp.astype(F8),
        "abt": abt,
    }
    return common


def kernel(**inputs):
    global LAST_RESULTS
    if "nc" not in _CACHE:
        _CACHE["nc"] = _build_program()
    nc = _CACHE["nc"]

    common = _prep_common(inputs)
    x = np.asarray(inputs["x"], np.float32)          # [8, 384, 28, 28]
    in_maps = []
    for c in range(B):
        m = dict(common)
        xc = x[c].reshape(3, 128, N).transpose(1, 0, 2).reshape(128, 3 * N)
        m["x_c"] = np.ascontiguousarray(xc).astype(BF)
        in_maps.append(m)

    import os
    trace = bool(int(os.environ.get("KERNEL_TRACE", "0")))
    res = run_bass_kernel_spmd(nc, in_maps, core_ids=list(range(B)),
                               trace=trace)
    LAST_RESULTS = res
    out = np.stack([res.results[c]["out"].reshape(DIM, RES, RES)
                    for c in range(B)])
    return out.astype(np.float32)


# revision 4
# speedup vs baseline: 1.0351x; 1.0253x over previous
"""Trainium2 Bass kernel for nn_Attention4D (EfficientViT-style attention).

Strategy (8 NeuronCores, data-parallel over batch B=8, one element per core):
  - BN folded into conv weights on host; talking-head-1 folded into
    per-head-scaled queries (Q2) so q@k contracts 256 channels.
  - Q2/K/e/w2s and the depthwise taps are fp8e4m3; the big matmuls use
    MatmulPerfMode.DoubleRow (2 K-tiles per instruction at 0.5 cycles/row):
      * logits: one DoubleRow (K=256) per 392-col chunk
      * rel-pos bias: DoubleRow with [I|0] weights
      * talking-head-2: produced TRANSPOSED as a2T = e_chunk^T @ w2s per
        (group, key-chunk) - keys land on partitions, so the DRAM transpose
        round-trip of the previous design disappears entirely.  Softmax
        normalization (1/z) rides inside w2s; bth2 is applied in phase D as
        a per-channel bias built from accum-reduced v sums.
      * depthwise 3x3: taps paired 2-per-DoubleRow with host-packed fp8
        diagonal weight pairs.
  - a2T / V stay bf16 (fp8 there fails the 2e-2 tolerance).
  - Phase C is software-pipelined; v/vt projections and phase-D head blocks
    are interleaved into the group loop as PE filler, with psum->sbuf copy
    work placed on ACT/DVE by a least-loaded heuristic.
"""

import sys

sys.path.insert(0, "/opt/trn_rl_repo")

import numpy as np
import ml_dtypes

import concourse.bass as bass
import concourse.tile as tile
from concourse import bacc, mybir
from concourse.bass_utils import run_bass_kernel_spmd
from bass_rust import AP

F32 = mybir.dt.float32
BF16 = mybir.dt.bfloat16
FP8 = mybir.dt.float8e4
AF = mybir.ActivationFunctionType
ALU = mybir.AluOpType
DR = mybir.MatmulPerfMode.DoubleRow
BF = ml_dtypes.bfloat16
F8 = ml_dtypes.float8_e4m3

HEADS, KD, AR, RES, DIM = 8, 32, 4, 28, 384
D = AR * KD            # 128
DH = HEADS * D         # 1024
NH_KD = HEADS * KD     # 256
N = RES * RES          # 784
NG = N // 16           # 49 groups of 16 queries
B = 8

_CACHE = {}
LAST_RESULTS = None  # test.py reads exec_time from here


def _sv(base, off_elems, dims):
    """Custom strided view: keep base AP's partition dim, free dims given as
    [(stride, size), ...]."""
    ap = [list(base.ap[0])] + [[s, n] for (s, n) in dims]
    return AP(base.tensor, base.offset + off_elems, ap)


def _build_program():
    nc = bacc.Bacc("TRN2", target_bir_lowering=False, debug=False,
                   enable_asserts=True)

    def din(name, shape, dt=F32):
        return nc.dram_tensor(name, shape, dt, kind="ExternalInput")

    x_c = din("x_c", [128, 3 * N], BF16)
    wq3 = din("wq3", [128, 3 * NH_KD], BF16)
    wk3 = din("wk3", [128, 3 * NH_KD], BF16)
    wv3 = din("wv3", [128, 3 * DH], BF16)
    wp8 = din("wp8", [128, 8 * DIM], BF16)
    vecs = din("vecs", [128, 48])
    w2bd = din("w2bd", [128, 128])
    idz = din("idz", [128, 256], FP8)
    dwp = din("dwp", [128, 8 * 9 * 128], FP8)
    abt = din("abt", [128, NG * N], FP8)

    out = nc.dram_tensor("out", [DIM, N], F32, kind="ExternalOutput")

    CH = (slice(0, 392), slice(392, 784))
    PSC = (slice(0, 392), slice(512, 904))

    def psum2view(ps):
        return ps[:].rearrange("p (a c) -> p a c", c=512)[:, :, 0:392]

    # engine-balance bookkeeping for flexible ACT/DVE ops
    ebusy = {"act": 0.0, "dve": 0.0}

    def flex(free, act_fn, dve_fn, accum=False):
        ca = (free + 250) * 0.833 + (187 if accum else 0)
        cd = (free + 150) * 1.042
        if ebusy["act"] + ca < ebusy["dve"] + cd:
            ebusy["act"] += ca
            act_fn()
        else:
            ebusy["dve"] += cd
            dve_fn()

    def act_only(free, fn, accum=False):
        ebusy["act"] += (free + 250) * 0.833 + (187 if accum else 0)
        fn()

    def dve_only(free, fn):
        ebusy["dve"] += (free + 150) * 1.042
        fn()

    with tile.TileContext(nc) as tc:
        with (
            tc.tile_pool(name="consts", bufs=1) as consts,
            tc.tile_pool(name="persist", bufs=1) as persist,
        ):
            vec_t = consts.tile([128, 48], F32, tag="vec_t", name="vec_t")
            bq_t = [vec_t[:, k:k + 1] for k in range(2)]
            bk_t = [vec_t[:, 2 + k:3 + k] for k in range(2)]
            bv_t = [vec_t[:, 4 + k:5 + k] for k in range(8)]
            bdw2_t = [vec_t[:, 12 + k:13 + k] for k in range(8)]
            bp_t = [vec_t[:, 20 + k:21 + k] for k in range(3)]
            bth1_t = vec_t[:, 23:24]
            bth2g_t = [vec_t[:, 24 + k:25 + k] for k in range(8)]
            sq_t = [[vec_t[:, 32 + kt * 8 + g:33 + kt * 8 + g]
                     for g in range(8)] for kt in range(2)]
            w2bd_t = consts.tile([128, 128], F32, tag="w2bd", name="w2bd")
            idz_t = consts.tile([128, 256], FP8, tag="idz", name="idz")
            dwp_t = consts.tile([128, 8 * 9 * 128], FP8, tag="dwp",
                                name="dwp")
            wp_w = consts.tile([128, 8 * DIM], BF16, tag="wp_w", name="wp_w")
            wp_t = [wp_w[:, k * DIM:(k + 1) * DIM] for k in range(8)]

            # persistent activations
            q2 = persist.tile([128, 2 * NG * 128], FP8, tag="q2", name="q2")
            q2tv = q2[:].rearrange("p (t c) -> p t c", c=NG * 128)
            k2 = persist.tile([128, 2 * N], FP8, tag="k2", name="k2")
            k2v = k2[:].rearrange("p (t c) -> p t c", c=N)
            vpad = [persist.tile([128, 900], FP8, tag=f"vpad{p}",
                                 name=f"vpad{p}") for p in range(8)]
            vt = [persist.tile([128, DH], BF16, tag=f"vt{m}", name=f"vt{m}")
                  for m in range(7)]
            a2t = persist.tile([128, 7 * NG * 128], BF16, tag="a2t",
                               name="a2t")
            a2tv = a2t[:].rearrange("p (m a c) -> p m a c", a=NG, c=128)
            osum = [persist.tile([128, N], BF16, tag=f"osum{p}",
                                 name=f"osum{p}") for p in range(8)]
            vsum = persist.tile([128, 8], F32, tag="vsum", name="vsum")
            bias2 = persist.tile([128, 8], F32, tag="bias2", name="bias2")
            qb = [persist.tile([128, N], BF16, tag=f"qb{k}", name=f"qb{k}")
                  for k in range(2)]

            with (
                tc.tile_pool(name="ax", bufs=1) as axpool,
                tc.tile_pool(name="pbig", bufs=2, space="PSUM") as pbig,
                tc.tile_pool(name="pa2", bufs=2, space="PSUM") as pa2,
                tc.tile_pool(name="cw", bufs=1) as cw,
                tc.tile_pool(name="cz", bufs=3) as cz,
            ):
                # ---------------- loads ---------------------------------
                x_w = axpool.tile([128, 3 * N], BF16, tag="xw", name="xw")
                for k in range(3):
                    nc.sync.dma_start(x_w[:, k * N:(k + 1) * N],
                                      x_c.ap()[:, k * N:(k + 1) * N])
                x_t = [x_w[:, k * N:(k + 1) * N] for k in range(3)]

                def load_a(name, src, shape, dt):
                    t = axpool.tile(shape, dt, tag=name, name=name)
                    nc.sync.dma_start(t[:], src)
                    return t

                wq_w = load_a("wq_w", wq3.ap()[:], [128, 3 * NH_KD], BF16)
                wk_w = load_a("wk_w", wk3.ap()[:], [128, 3 * NH_KD], BF16)
                wv_w = load_a("wv_w", wv3.ap()[:], [128, 3 * DH], BF16)
                wq_t = [wq_w[:, k * NH_KD:(k + 1) * NH_KD] for k in range(3)]
                wk_t = [wk_w[:, k * NH_KD:(k + 1) * NH_KD] for k in range(3)]
                wv_t = [wv_w[:, k * DH:(k + 1) * DH] for k in range(3)]
                nc.sync.dma_start(vec_t[:], vecs.ap()[:])
                nc.sync.dma_start(w2bd_t[:], w2bd.ap()[:])
                nc.sync.dma_start(idz_t[:], idz.ap()[:])
                nc.sync.dma_start(dwp_t[:], dwp.ap()[:])

                # q2 fp8 prep, batched by group ranges; batch 0 is
                # emitted inside the projection loop right after each qb
                # copy so Pool starts on kt0 while kt1 is still projecting
                q2_batches = [(0, 8), (8, 21), (21, 34), (34, 49)]
                q2_ops = []
                for (a0, a1) in q2_batches[1:]:
                    for kt in range(2):
                        for g in range(8):
                            q2_ops.append((kt, g, a0, a1))

                def emit_q2(kt, g, a0, a1):
                    dst = _sv(q2[:], kt * NG * 128 + a0 * 128 + g * 16,
                              [(128, a1 - a0), (1, 16)])
                    src = _sv(qb[kt][:], a0 * 16, [(16, a1 - a0), (1, 16)])
                    nc.gpsimd.tensor_scalar_mul(dst, src, sq_t[kt][g])

                # ---------------- q/k projections ------------------------
                for (wts, bias, fp8out) in ((wq_t, bq_t, False),
                                            (wk_t, bk_t, True)):
                    for ot in range(2):
                        ps = pbig.tile([128, 1024], F32, tag="big",
                                       name="big")
                        for ci in range(2):
                            pchunk = ps[:, PSC[ci]]
                            for kt in range(3):
                                nc.tensor.matmul(
                                    pchunk,
                                    lhsT=wts[kt][:, ot * 128:(ot + 1) * 128],
                                    rhs=x_t[kt][:, CH[ci]],
                                    start=(kt == 0), stop=(kt == 2))
                        if fp8out:
                            dst = k2v[:, ot, :].rearrange(
                                "p (a c) -> p a c", c=392)
                        else:
                            dst = qb[ot][:].rearrange(
                                "p (a c) -> p a c", c=392)
                        dve_only(784, lambda d=dst, p=ps, b=bias[ot]:
                                 nc.vector.tensor_scalar_add(
                                     d, psum2view(p), b))
                        if not fp8out:
                            for g in range(8):
                                emit_q2(ot, g, 0, 8)

                # ---------------- filler task queues ---------------------
                def v_task(p):
                    def go():
                        vvz = vpad[p][:].rearrange("p (r c) -> p r c", c=30)
                        nc.gpsimd.memset(vvz[:, 0, :], 0.0)
                        nc.gpsimd.memset(vvz[:, 29, :], 0.0)
                        nc.gpsimd.memset(vvz[:, 1:29, 0], 0.0)
                        nc.gpsimd.memset(vvz[:, 1:29, 29], 0.0)
                        ps = pa2.tile([128, 1024], F32, tag="a2ps",
                                      name="a2ps")
                        for ci in range(2):
                            for kt in range(3):
                                nc.tensor.matmul(
                                    ps[:, PSC[ci]],
                                    lhsT=wv_t[kt][:, p * 128:(p + 1) * 128],
                                    rhs=x_t[kt][:, CH[ci]],
                                    start=(kt == 0), stop=(kt == 2))
                        rows = vvz[:, 1:29, 1:29].rearrange(
                            "p (a r) c -> p a r c", a=2)
                        pin = psum2view(ps).rearrange(
                            "p a (r c) -> p a r c", c=28)
                        vs = vsum[:, p:p + 1]
                        flex(784,
                             lambda: nc.scalar.activation(
                                 rows, pin, AF.Identity, bias=bv_t[p],
                                 accum_out=vs),
                             lambda: nc.vector.tensor_scalar(
                                 rows, pin, bv_t[p], 0.0, ALU.add,
                                 ALU.add, accum_out=vs),
                             accum=True)
                        if p == 7:
                            for g in range(8):
                                nc.vector.scalar_tensor_tensor(
                                    bias2[:, g:g + 1], vsum[:, g:g + 1],
                                    bth2g_t[g], bdw2_t[g],
                                    ALU.mult, ALU.add)
                    return go

                def vt_task(mt):
                    def go():
                        M = 128 if mt < 6 else 16
                        if mt == 6:
                            nc.gpsimd.memset(vt[6][:], 0.0)
                        msl = slice(mt * 128, mt * 128 + M)
                        ps = pa2.tile([128, 1024], F32, tag="a2ps",
                                      name="a2ps")
                        for ci in range(2):
                            pchunk = ps[0:M, ci * 512:(ci + 1) * 512]
                            wsl = slice(ci * 512, (ci + 1) * 512)
                            for kt in range(3):
                                nc.tensor.matmul(pchunk,
                                                 lhsT=x_t[kt][:, msl],
                                                 rhs=wv_t[kt][:, wsl],
                                                 start=(kt == 0),
                                                 stop=(kt == 2))
                        flex(1024,
                             lambda p=ps, m=M:
                             nc.scalar.copy(vt[mt][0:m, :], p[0:m, :]),
                             lambda p=ps, m=M:
                             nc.vector.tensor_scalar_add(
                                 vt[mt][0:m, :], p[0:m, :], 0.0))
                    return go

                DCH = ((0, 16, 0, 28, 448), (16, 12, 28, NG, 336))

                def dw_attnv(g, ci, pool, tag, tilew=1024):
                    r0, nr, a0, a1, w = DCH[ci]
                    csl = slice(r0 * 28, r0 * 28 + w)
                    po = pool.tile([128, tilew], F32, tag=tag, name=tag)
                    pov = po[:, 0:w]
                    vflat = vpad[g][:]
                    for p in range(4):
                        t0 = 2 * p
                        dy0, dx0 = t0 // 3, t0 % 3
                        dy1, dx1 = (t0 + 1) // 3, (t0 + 1) % 3
                        delta = (dy1 - dy0) * 30 + (dx1 - dx0)
                        rhs = _sv(vflat, (r0 + dy0) * 30 + dx0,
                                  [(delta, 2), (30, nr), (1, 28)])
                        lw = dwp_t[:, g * 1152 + p * 256:
                                   g * 1152 + p * 256 + 256].rearrange(
                                       "p (t c) -> p t c", c=128)
                        nc.tensor.matmul(pov, lhsT=lw, rhs=rhs,
                                         start=(p == 0), stop=False,
                                         perf_mode=DR)
                    rhs8 = _sv(vflat, (r0 + 2) * 30 + 2,
                               [(30, nr), (1, 28)])
                    nc.tensor.matmul(
                        pov,
                        lhsT=dwp_t[:, g * 1152 + 1024:g * 1152 + 1152],
                        rhs=rhs8, start=False, stop=False)
                    for mt in range(7):
                        cols = a2tv[:, mt, a0:a1, g * 16:(g + 1) * 16]
                        nc.tensor.matmul(
                            pov, lhsT=vt[mt][:, g * 128:(g + 1) * 128],
                            rhs=cols, start=False, stop=(mt == 6))
                    flex(w,
                         lambda: nc.scalar.activation(
                             osum[g][:, csl], pov, AF.Identity,
                             bias=bias2[:, g:g + 1]),
                         lambda: nc.vector.tensor_scalar_add(
                             osum[g][:, csl], pov, bias2[:, g:g + 1]))

                tasks_early = [v_task(p) for p in range(8)] + \
                              [vt_task(mt) for mt in range(7)]
                tasks_late = [(lambda gg=g: dw_attnv(gg, 0, pa2, "a2ps"))
                              for g in range(8)]

                # ---------------- phase C group loop ---------------------
                abtiles = {}

                def fetch_ab(kb):
                    nab = min(4, NG - kb * 4)
                    t = cw.tile([128, 4 * N + 392], FP8, tag="ab", name="ab",
                                bufs=5)
                    if kb < 5:
                        nc.gpsimd.memset(t[:, 4 * N:], 0.0)
                    nc.sync.dma_start(
                        t[:, 0:nab * N],
                        abt.ap()[:, kb * 4 * N:(kb * 4 + nab) * N])
                    abtiles[kb] = t

                for _k in range(4):
                    fetch_ab(_k)

                pending = []

                def th2t_and_copy(gi, e_t, w2s_t):
                    a2ps = pa2.tile([128, 1024], F32, tag="a2ps",
                                    name="a2ps")
                    w2sv = w2s_t[:].rearrange("p (t c) -> p t c", c=128)
                    for m in range(7):
                        # k-tile pair [e chunk; zeros at col 896]
                        lhsT = _sv(e_t[:], m * 128,
                                   [(896 - m * 128, 2), (1, 128)])
                        nc.tensor.matmul(
                            a2ps[:, m * 128:(m + 1) * 128],
                            lhsT=lhsT, rhs=w2sv,
                            start=True, stop=True, perf_mode=DR)
                    src = a2ps[:, 0:896].rearrange("p (m c) -> p m c",
                                                   c=128)
                    dst = a2tv[:, :, gi, :]
                    flex(896,
                         lambda: nc.scalar.copy(dst, src),
                         lambda: nc.vector.tensor_scalar_add(dst, src, 0.0))

                for gi in range(NG):
                    if gi % 4 == 2 and gi // 4 + 4 <= (NG - 1) // 4:
                        fetch_ab(gi // 4 + 4)
                    ab4 = abtiles[gi // 4]
                    j = gi % 4

                    lg = pbig.tile([128, 1024], F32, tag="big", name="big")
                    for ci in range(2):
                        pchunk = lg[:, PSC[ci]]
                        nc.tensor.matmul(
                            pchunk, lhsT=q2tv[:, :, gi * 128:(gi + 1) * 128],
                            rhs=k2v[:, :, CH[ci]],
                            start=True, stop=False, perf_mode=DR)
                        abrhs = _sv(ab4[:], j * N + ci * 392,
                                    [(392, 2), (1, 392)])
                        nc.tensor.matmul(
                            pchunk,
                            lhsT=idz_t[:].rearrange("p (t c) -> p t c",
                                                    c=128),
                            rhs=abrhs,
                            start=False, stop=True, perf_mode=DR)

                    e_t = cw.tile([128, 1024], FP8, tag="e", name="e",
                                  bufs=4)
                    if gi < 4:
                        nc.gpsimd.memset(e_t[:, 784:1024], 0.0)
                    z = cz.tile([128, 1], F32, tag="z", name="z")
                    ev = e_t[:, 0:784].rearrange("p (a c) -> p a c", c=392)
                    act_only(784, lambda: nc.scalar.activation(
                        ev, psum2view(lg), AF.Exp, bias=bth1_t,
                        accum_out=z[:]), accum=True)

                    w2s_t = cw.tile([128, 256], FP8, tag="w2s", name="w2s",
                                    bufs=4)
                    if gi < 4:
                        nc.gpsimd.memset(w2s_t[:, 128:256], 0.0)
                    r = cz.tile([128, 1], F32, tag="r", name="r")
                    nc.vector.reciprocal(r[:], z[:])
                    dve_only(128, lambda: nc.vector.tensor_scalar_mul(
                        w2s_t[:, 0:128], w2bd_t[:], r[:]))

                    pending.append((gi, e_t, w2s_t))
                    if len(pending) > 2:
                        th2t_and_copy(*pending.pop(0))

                    # fillers
                    if gi < 28:
                        if tasks_early and (gi * 15) // 28 > \
                                14 - len(tasks_early):
                            tasks_early.pop(0)()
                    elif gi >= 30 and tasks_late and (gi - 30) % 2 == 0:
                        tasks_late.pop(0)()
                    for _ in range(2):
                        if q2_ops:
                            emit_q2(*q2_ops.pop(0))

                while pending:
                    th2t_and_copy(*pending.pop(0))
                while tasks_early:
                    tasks_early.pop(0)()
                while tasks_late:
                    tasks_late.pop(0)()

            # ================= phase D tail ==========================
            with (
                tc.tile_pool(name="pd", bufs=2, space="PSUM") as pd,
                tc.tile_pool(name="pe", bufs=1, space="PSUM") as pe,
                tc.tile_pool(name="ow", bufs=1) as ow,
            ):
                nc.sync.dma_start(wp_w[:], wp8.ap()[:])
                ot = [ow.tile([128, N], F32, tag=f"ot{mt}", name=f"ot{mt}")
                      for mt in range(3)]

                # chunk-0 p-projection first (all osum chunk-0 ready)
                pp0 = [pe.tile([128, 448], F32, tag=f"pp0{mt}",
                               name=f"pp0{mt}") for mt in range(3)]
                for g in range(8):
                    for mt in range(3):
                        nc.tensor.matmul(
                            pp0[mt][:],
                            lhsT=wp_t[g][:, mt * 128:(mt + 1) * 128],
                            rhs=osum[g][:, 0:448],
                            start=(g == 0), stop=(g == 7))
                for mt in range(3):
                    flex(448, lambda m=mt: nc.scalar.activation(
                        ot[m][:, 0:448], pp0[m][:], AF.Identity,
                        bias=bp_t[m]),
                        lambda m=mt: nc.vector.tensor_scalar_add(
                            ot[m][:, 0:448], pp0[m][:], bp_t[m]))
                    nc.sync.dma_start(out.ap()[mt * 128:(mt + 1) * 128,
                                               0:448], ot[mt][:, 0:448])

                # D1 heads with chunk-1 p-projection pipelined one head back
                pp1 = [pe.tile([128, 336], F32, tag=f"pp1{mt}",
                               name=f"pp1{mt}") for mt in range(3)]

                def pp1_g(g):
                    for mt in range(3):
                        nc.tensor.matmul(
                            pp1[mt][:],
                            lhsT=wp_t[g][:, mt * 128:(mt + 1) * 128],
                            rhs=osum[g][:, 448:784],
                            start=(g == 0), stop=(g == 7))

                for g in range(8):
                    dw_attnv(g, 1, pd, "po2", tilew=448)
                    if g >= 1:
                        pp1_g(g - 1)
                pp1_g(7)
                for mt in range(3):
                    if mt == 1:
                        nc.vector.tensor_scalar_add(
                            ot[mt][:, 448:784], pp1[mt][:], bp_t[mt])
                    else:
                        nc.scalar.activation(
                            ot[mt][:, 448:784], pp1[mt][:], AF.Identity,
                            bias=bp_t[mt])
                    trig = nc.gpsimd if mt == 1 else nc.sync
                    trig.dma_start(out.ap()[mt * 128:(mt + 1) * 128,
                                            448:784],
                                   ot[mt][:, 448:784])

    nc.compile()
    return nc


def _prep_common(inputs):
    f32 = np.float32
    scale = np.float32(KD ** -0.5)
    q_s, q_b = inputs["q_s"], inputs["q_b"]
    k_s, k_b = inputs["k_s"], inputs["k_b"]
    v_s, v_b = inputs["v_s"], inputs["v_b"]
    p_s, p_b = inputs["p_s"], inputs["p_b"]

    Wq = np.asarray(inputs["Wq"], f32) * np.asarray(q_s, f32)[:, None] * scale
    bqv = (np.asarray(q_s, f32) * np.asarray(inputs["bq"], f32)
           + np.asarray(q_b, f32)) * scale
    Wk = np.asarray(inputs["Wk"], f32) * np.asarray(k_s, f32)[:, None]
    bkv = np.asarray(k_s, f32) * np.asarray(inputs["bk"], f32) \
        + np.asarray(k_b, f32)
    Wv = np.asarray(inputs["Wv"], f32) * np.asarray(v_s, f32)[:, None]
    bvv = np.asarray(v_s, f32) * np.asarray(inputs["bv"], f32) \
        + np.asarray(v_b, f32)
    Wp = np.asarray(inputs["Wp"], f32) * np.asarray(p_s, f32)[:, None]
    bpv = np.asarray(p_s, f32) * np.asarray(inputs["bp"], f32) \
        + np.asarray(p_b, f32)

    Wth1 = np.asarray(inputs["Wth1"], f32)
    bth1 = np.asarray(inputs["bth1"], f32)
    Wth2 = np.asarray(inputs["Wth2"], f32)
    bth2 = np.asarray(inputs["bth2"], f32)

    # th1-folded rel-pos bias table, rows (group, g, i)
    ab1 = Wth1 @ np.asarray(inputs["attention_biases"], f32)
    idx = np.asarray(inputs["bias_idxs"])
    ab_full = ab1[:, idx]                                  # [8,784,784]
    # rows (group, g, i) -> partition-major [128, NG*784] so each 4-group
    # fetch is one contiguous segment per partition
    abt = np.ascontiguousarray(
        ab_full.reshape(8, NG, 16, N).transpose(1, 0, 2, 3)
    ).reshape(NG, 128, N).transpose(1, 0, 2).reshape(128, NG * N)
    abt = np.ascontiguousarray(abt).astype(F8)

    # depthwise weights folded with BN -> fp8 diagonal pair blocks
    wvl = np.asarray(inputs["Wvl"], f32)[:, 0, :, :].reshape(DH, 9)
    vl_s = np.asarray(inputs["vl_s"], f32)
    wtap = wvl * vl_s[:, None]                             # [1024, 9]
    bdw = (np.asarray(inputs["bvl"], f32) * vl_s
           + np.asarray(inputs["vl_b"], f32))
    dwp = np.zeros((128, 8 * 9 * 128), f32)
    for g in range(8):
        for t in range(9):
            blk = np.zeros((128, 128), f32)
            np.fill_diagonal(blk, wtap[g * 128:(g + 1) * 128, t])
            dwp[:, g * 1152 + t * 128:g * 1152 + (t + 1) * 128] = blk

    def ktile_pack(wT, nk):
        C = wT.shape[1]
        return np.ascontiguousarray(
            wT.reshape(nk, 128, C).transpose(1, 0, 2).reshape(128, nk * C))

    sqv = np.repeat(Wth1.T, KD, axis=0).astype(f32)        # [256, 8]
    vecs = np.zeros((128, 48), f32)
    vecs[:, 0:2] = bqv.reshape(2, 128).T
    vecs[:, 2:4] = bkv.reshape(2, 128).T
    vecs[:, 4:12] = bvv.reshape(8, 128).T
    s2p = Wth2.sum(axis=1)                                 # no N*bth2 term
    bdw2 = bdw + bvv * np.repeat(s2p, D)
    vecs[:, 12:20] = bdw2.reshape(8, 128).T
    vecs[:, 20:23] = bpv.reshape(3, 128).T
    vecs[:, 23] = np.repeat(bth1, 16)
    for g in range(8):
        vecs[:, 24 + g] = bth2[g]
    vecs[:, 32:40] = sqv[0:128]
    vecs[:, 40:48] = sqv[128:256]

    common = {
        "wq3": ktile_pack(np.ascontiguousarray(Wq.T), 3).astype(BF),
        "wk3": ktile_pack(np.ascontiguousarray(Wk.T), 3).astype(BF),
        "wv3": ktile_pack(np.ascontiguousarray(Wv.T), 3).astype(BF),
        "wp8": ktile_pack(np.ascontiguousarray(Wp.T), 8).astype(BF),
        "vecs": vecs,
        "w2bd": np.kron(Wth2.T, np.eye(16, dtype=f32)).astype(f32),
        "idz": np.concatenate([np.eye(128, dtype=f32),
                               np.zeros((128, 128), f32)],
                              axis=1).astype(F8),
        "dwp": dwp.astype(F8),
        "abt": abt,
    }
    return common


def kernel(**inputs):
    global LAST_RESULTS
    if "nc" not in _CACHE:
        _CACHE["nc"] = _build_program()
    nc = _CACHE["nc"]

    common = _prep_common(inputs)
    x = np.asarray(inputs["x"], np.float32)          # [8, 384, 28, 28]
    in_maps = []
    for c in range(B):
        m = dict(common)
        xc = x[c].reshape(3, 128, N).transpose(1, 0, 2).reshape(128, 3 * N)
        m["x_c"] = np.ascontiguousarray(xc).astype(BF)
        in_maps.append(m)

    import os
    trace = bool(int(os.environ.get("KERNEL_TRACE", "0")))
    res = run_bass_kernel_spmd(nc, in_maps, core_ids=list(range(B)),
                               trace=trace)
    LAST_RESULTS = res
    out = np.stack([res.results[c]["out"].reshape(DIM, RES, RES)
                    for c in range(B)])
    return out.astype(np.float32)


# revision 5
# speedup vs baseline: 1.0777x; 1.0411x over previous
"""Trainium2 Bass kernel for nn_Attention4D (EfficientViT-style attention).

Strategy (8 NeuronCores, data-parallel over batch B=8, one element per core):
  - BN folded into conv weights on host; talking-head-1 folded into
    per-head-scaled queries (Q2) so q@k contracts 256 channels.
  - Q2/K/e/w2s and the depthwise taps are fp8e4m3; the big matmuls use
    MatmulPerfMode.DoubleRow (2 K-tiles per instruction at 0.5 cycles/row):
      * logits: one DoubleRow (K=256) per 392-col chunk
      * rel-pos bias: DoubleRow with [I|0] weights
      * talking-head-2: produced TRANSPOSED as a2T = e_chunk^T @ w2s per
        (group, key-chunk) - keys land on partitions, so the DRAM transpose
        round-trip of the previous design disappears entirely.  Softmax
        normalization (1/z) rides inside w2s; bth2 is applied in phase D as
        a per-channel bias built from accum-reduced v sums.
      * depthwise 3x3: taps paired 2-per-DoubleRow with host-packed fp8
        diagonal weight pairs.
  - a2T / V stay bf16 (fp8 there fails the 2e-2 tolerance).
  - Phase C is software-pipelined; v/vt projections and phase-D head blocks
    are interleaved into the group loop as PE filler, with psum->sbuf copy
    work placed on ACT/DVE by a least-loaded heuristic.
"""

import sys

sys.path.insert(0, "/opt/trn_rl_repo")

import numpy as np
import ml_dtypes

import concourse.bass as bass
import concourse.tile as tile
from concourse import bacc, mybir
from concourse.bass_utils import run_bass_kernel_spmd
from bass_rust import AP

F32 = mybir.dt.float32
BF16 = mybir.dt.bfloat16
FP8 = mybir.dt.float8e4
AF = mybir.ActivationFunctionType
ALU = mybir.AluOpType
DR = mybir.MatmulPerfMode.DoubleRow
BF = ml_dtypes.bfloat16
F8 = ml_dtypes.float8_e4m3

HEADS, KD, AR, RES, DIM = 8, 32, 4, 28, 384
D = AR * KD            # 128
DH = HEADS * D         # 1024
NH_KD = HEADS * KD     # 256
N = RES * RES          # 784
NG = N // 16           # 49 groups of 16 queries
B = 8

_CACHE = {}
LAST_RESULTS = None  # test.py reads exec_time from here


def _sv(base, off_elems, dims):
    """Custom strided view: keep base AP's partition dim, free dims given as
    [(stride, size), ...]."""
    ap = [list(base.ap[0])] + [[s, n] for (s, n) in dims]
    return AP(base.tensor, base.offset + off_elems, ap)


def _build_program():
    nc = bacc.Bacc("TRN2", target_bir_lowering=False, debug=False,
                   enable_asserts=True)

    def din(name, shape, dt=F32):
        return nc.dram_tensor(name, shape, dt, kind="ExternalInput")

    x_c = din("x_c", [128, 3 * N], BF16)
    wq3 = din("wq3", [128, 3 * NH_KD], BF16)
    wk3 = din("wk3", [128, 3 * NH_KD], BF16)
    wv3 = din("wv3", [128, 3 * DH], BF16)
    wp8 = din("wp8", [128, 8 * DIM], BF16)
    vecs = din("vecs", [128, 48])
    w2bd = din("w2bd", [128, 128])
    idz = din("idz", [128, 256], FP8)
    dwp = din("dwp", [128, 8 * 9 * 128], FP8)
    abt = din("abt", [128, NG * N], FP8)

    out = nc.dram_tensor("out", [DIM, N], F32, kind="ExternalOutput")

    CH = (slice(0, 392), slice(392, 784))
    PSC = (slice(0, 392), slice(512, 904))

    def psum2view(ps):
        return ps[:].rearrange("p (a c) -> p a c", c=512)[:, :, 0:392]

    # engine-balance bookkeeping for flexible ACT/DVE ops
    ebusy = {"act": 0.0, "dve": 0.0}

    def flex(free, act_fn, dve_fn, accum=False):
        ca = (free + 250) * 0.833 + (187 if accum else 0)
        cd = (free + 150) * 1.042
        if ebusy["act"] + ca < ebusy["dve"] + cd:
            ebusy["act"] += ca
            act_fn()
        else:
            ebusy["dve"] += cd
            dve_fn()

    def act_only(free, fn, accum=False):
        ebusy["act"] += (free + 250) * 0.833 + (187 if accum else 0)
        fn()

    def dve_only(free, fn):
        ebusy["dve"] += (free + 150) * 1.042
        fn()

    with tile.TileContext(nc) as tc:
        with (
            tc.tile_pool(name="consts", bufs=1) as consts,
            tc.tile_pool(name="persist", bufs=1) as persist,
        ):
            vec_t = consts.tile([128, 48], F32, tag="vec_t", name="vec_t")
            bq_t = [vec_t[:, k:k + 1] for k in range(2)]
            bk_t = [vec_t[:, 2 + k:3 + k] for k in range(2)]
            bv_t = [vec_t[:, 4 + k:5 + k] for k in range(8)]
            bdw2_t = [vec_t[:, 12 + k:13 + k] for k in range(8)]
            bp_t = [vec_t[:, 20 + k:21 + k] for k in range(3)]
            bth1_t = vec_t[:, 23:24]
            bth2g_t = [vec_t[:, 24 + k:25 + k] for k in range(8)]
            sq_t = [[vec_t[:, 32 + kt * 8 + g:33 + kt * 8 + g]
                     for g in range(8)] for kt in range(2)]
            w2bd_t = consts.tile([128, 128], F32, tag="w2bd", name="w2bd")
            idz_t = consts.tile([128, 256], FP8, tag="idz", name="idz")
            dwp_t = consts.tile([128, 8 * 9 * 128], FP8, tag="dwp",
                                name="dwp")
            wp_w = consts.tile([128, 8 * DIM], BF16, tag="wp_w", name="wp_w")
            wp_t = [wp_w[:, k * DIM:(k + 1) * DIM] for k in range(8)]

            # persistent activations
            q2 = persist.tile([128, 2 * NG * 128], FP8, tag="q2", name="q2")
            q2tv = q2[:].rearrange("p (t c) -> p t c", c=NG * 128)
            k2 = persist.tile([128, 2 * N], FP8, tag="k2", name="k2")
            k2v = k2[:].rearrange("p (t c) -> p t c", c=N)
            vpad = [persist.tile([128, 900], FP8, tag=f"vpad{p}",
                                 name=f"vpad{p}") for p in range(8)]
            vt = [persist.tile([128, DH], BF16, tag=f"vt{m}", name=f"vt{m}")
                  for m in range(7)]
            a2t = persist.tile([128, 7 * NG * 128], BF16, tag="a2t",
                               name="a2t")
            a2tv = a2t[:].rearrange("p (m a c) -> p m a c", a=NG, c=128)
            osum = [persist.tile([128, N], BF16, tag=f"osum{p}",
                                 name=f"osum{p}") for p in range(8)]
            vsum = persist.tile([128, 8], F32, tag="vsum", name="vsum")
            bias2 = persist.tile([128, 8], F32, tag="bias2", name="bias2")
            qb = [persist.tile([128, N], BF16, tag=f"qb{k}", name=f"qb{k}")
                  for k in range(2)]

            with (
                tc.tile_pool(name="ax", bufs=1) as axpool,
                tc.tile_pool(name="pbig", bufs=2, space="PSUM") as pbig,
                tc.tile_pool(name="pa2", bufs=2, space="PSUM") as pa2,
                tc.tile_pool(name="cw", bufs=1) as cw,
                tc.tile_pool(name="cz", bufs=3) as cz,
            ):
                # ---------------- loads ---------------------------------
                x_w = axpool.tile([128, 3 * N], BF16, tag="xw", name="xw")
                for k in range(3):
                    nc.sync.dma_start(x_w[:, k * N:(k + 1) * N],
                                      x_c.ap()[:, k * N:(k + 1) * N])
                x_t = [x_w[:, k * N:(k + 1) * N] for k in range(3)]

                def load_a(name, src, shape, dt):
                    t = axpool.tile(shape, dt, tag=name, name=name)
                    nc.sync.dma_start(t[:], src)
                    return t

                wq_w = load_a("wq_w", wq3.ap()[:], [128, 3 * NH_KD], BF16)
                wk_w = load_a("wk_w", wk3.ap()[:], [128, 3 * NH_KD], BF16)
                wv_w = load_a("wv_w", wv3.ap()[:], [128, 3 * DH], BF16)
                wq_t = [wq_w[:, k * NH_KD:(k + 1) * NH_KD] for k in range(3)]
                wk_t = [wk_w[:, k * NH_KD:(k + 1) * NH_KD] for k in range(3)]
                wv_t = [wv_w[:, k * DH:(k + 1) * DH] for k in range(3)]
                nc.sync.dma_start(vec_t[:], vecs.ap()[:])
                nc.sync.dma_start(w2bd_t[:], w2bd.ap()[:])
                nc.sync.dma_start(idz_t[:], idz.ap()[:])
                nc.sync.dma_start(dwp_t[:], dwp.ap()[:])

                # q2 fp8 prep, batched by group ranges; batch 0 is
                # emitted inside the projection loop right after each qb
                # copy so Pool starts on kt0 while kt1 is still projecting
                q2_batches = [(0, 8), (8, 21), (21, 34), (34, 49)]
                q2_ops = []
                for (a0, a1) in q2_batches[1:]:
                    for kt in range(2):
                        for g in range(8):
                            q2_ops.append((kt, g, a0, a1))

                def emit_q2(kt, g, a0, a1):
                    dst = _sv(q2[:], kt * NG * 128 + a0 * 128 + g * 16,
                              [(128, a1 - a0), (1, 16)])
                    src = _sv(qb[kt][:], a0 * 16, [(16, a1 - a0), (1, 16)])
                    nc.gpsimd.tensor_scalar_mul(dst, src, sq_t[kt][g])

                # ---------------- q/k projections ------------------------
                for (wts, bias, fp8out) in ((wq_t, bq_t, False),
                                            (wk_t, bk_t, True)):
                    for ot in range(2):
                        ps = pbig.tile([128, 1024], F32, tag="big",
                                       name="big")
                        for ci in range(2):
                            pchunk = ps[:, PSC[ci]]
                            for kt in range(3):
                                nc.tensor.matmul(
                                    pchunk,
                                    lhsT=wts[kt][:, ot * 128:(ot + 1) * 128],
                                    rhs=x_t[kt][:, CH[ci]],
                                    start=(kt == 0), stop=(kt == 2))
                        if fp8out:
                            dst = k2v[:, ot, :].rearrange(
                                "p (a c) -> p a c", c=392)
                        else:
                            dst = qb[ot][:].rearrange(
                                "p (a c) -> p a c", c=392)
                        dve_only(784, lambda d=dst, p=ps, b=bias[ot]:
                                 nc.vector.tensor_scalar_add(
                                     d, psum2view(p), b))
                        if not fp8out:
                            for g in range(8):
                                emit_q2(ot, g, 0, 8)

                # ---------------- filler task queues ---------------------
                def v_task(p):
                    def go():
                        vvz = vpad[p][:].rearrange("p (r c) -> p r c", c=30)
                        nc.gpsimd.memset(vvz[:, 0, :], 0.0)
                        nc.gpsimd.memset(vvz[:, 29, :], 0.0)
                        nc.gpsimd.memset(vvz[:, 1:29, 0], 0.0)
                        nc.gpsimd.memset(vvz[:, 1:29, 29], 0.0)
                        ps = pa2.tile([128, 1024], F32, tag="a2ps",
                                      name="a2ps")
                        for ci in range(2):
                            for kt in range(3):
                                nc.tensor.matmul(
                                    ps[:, PSC[ci]],
                                    lhsT=wv_t[kt][:, p * 128:(p + 1) * 128],
                                    rhs=x_t[kt][:, CH[ci]],
                                    start=(kt == 0), stop=(kt == 2))
                        rows = vvz[:, 1:29, 1:29].rearrange(
                            "p (a r) c -> p a r c", a=2)
                        pin = psum2view(ps).rearrange(
                            "p a (r c) -> p a r c", c=28)
                        vs = vsum[:, p:p + 1]
                        flex(784,
                             lambda: nc.scalar.activation(
                                 rows, pin, AF.Identity, bias=bv_t[p],
                                 accum_out=vs),
                             lambda: nc.vector.tensor_scalar(
                                 rows, pin, bv_t[p], 0.0, ALU.add,
                                 ALU.add, accum_out=vs),
                             accum=True)
                        if p == 7:
                            for g in range(8):
                                nc.vector.scalar_tensor_tensor(
                                    bias2[:, g:g + 1], vsum[:, g:g + 1],
                                    bth2g_t[g], bdw2_t[g],
                                    ALU.mult, ALU.add)
                    return go

                def vt_task(mt):
                    def go():
                        M = 128 if mt < 6 else 16
                        if mt == 6:
                            nc.gpsimd.memset(vt[6][:], 0.0)
                        msl = slice(mt * 128, mt * 128 + M)
                        ps = pa2.tile([128, 1024], F32, tag="a2ps",
                                      name="a2ps")
                        for ci in range(2):
                            pchunk = ps[0:M, ci * 512:(ci + 1) * 512]
                            wsl = slice(ci * 512, (ci + 1) * 512)
                            for kt in range(3):
                                nc.tensor.matmul(pchunk,
                                                 lhsT=x_t[kt][:, msl],
                                                 rhs=wv_t[kt][:, wsl],
                                                 start=(kt == 0),
                                                 stop=(kt == 2))
                        flex(1024,
                             lambda p=ps, m=M:
                             nc.scalar.copy(vt[mt][0:m, :], p[0:m, :]),
                             lambda p=ps, m=M:
                             nc.vector.tensor_scalar_add(
                                 vt[mt][0:m, :], p[0:m, :], 0.0))
                    return go

                DCH = ((0, 16, 0, 28, 448), (16, 12, 28, NG, 336))

                def dw_attnv(g, ci, pool, tag, tilew=1024):
                    r0, nr, a0, a1, w = DCH[ci]
                    csl = slice(r0 * 28, r0 * 28 + w)
                    po = pool.tile([128, tilew], F32, tag=tag, name=tag)
                    pov = po[:, 0:w]
                    vflat = vpad[g][:]
                    for p in range(4):
                        t0 = 2 * p
                        dy0, dx0 = t0 // 3, t0 % 3
                        dy1, dx1 = (t0 + 1) // 3, (t0 + 1) % 3
                        delta = (dy1 - dy0) * 30 + (dx1 - dx0)
                        rhs = _sv(vflat, (r0 + dy0) * 30 + dx0,
                                  [(delta, 2), (30, nr), (1, 28)])
                        lw = dwp_t[:, g * 1152 + p * 256:
                                   g * 1152 + p * 256 + 256].rearrange(
                                       "p (t c) -> p t c", c=128)
                        nc.tensor.matmul(pov, lhsT=lw, rhs=rhs,
                                         start=(p == 0), stop=False,
                                         perf_mode=DR)
                    rhs8 = _sv(vflat, (r0 + 2) * 30 + 2,
                               [(30, nr), (1, 28)])
                    nc.tensor.matmul(
                        pov,
                        lhsT=dwp_t[:, g * 1152 + 1024:g * 1152 + 1152],
                        rhs=rhs8, start=False, stop=False)
                    for mt in range(7):
                        cols = a2tv[:, mt, a0:a1, g * 16:(g + 1) * 16]
                        nc.tensor.matmul(
                            pov, lhsT=vt[mt][:, g * 128:(g + 1) * 128],
                            rhs=cols, start=False, stop=(mt == 6))
                    flex(w,
                         lambda: nc.scalar.activation(
                             osum[g][:, csl], pov, AF.Identity,
                             bias=bias2[:, g:g + 1]),
                         lambda: nc.vector.tensor_scalar_add(
                             osum[g][:, csl], pov, bias2[:, g:g + 1]))

                tasks_early = [v_task(p) for p in range(8)] + \
                              [vt_task(mt) for mt in range(7)]
                tasks_late = [(lambda gg=g: dw_attnv(gg, 0, pa2, "a2ps"))
                              for g in range(8)]

                # ---------------- phase C group loop ---------------------
                abtiles = {}

                def fetch_ab(kb):
                    nab = min(8, NG - kb * 8)
                    t = cw.tile([128, 8 * N + 392], FP8, tag="ab", name="ab",
                                bufs=3)
                    if kb < 3:
                        nc.gpsimd.memset(t[:, 8 * N:], 0.0)
                    nc.sync.dma_start(
                        t[:, 0:nab * N],
                        abt.ap()[:, kb * 8 * N:(kb * 8 + nab) * N])
                    abtiles[kb] = t

                for _k in range(2):
                    fetch_ab(_k)

                pending = []

                def th2t_and_copy(gi, e_t, w2s_t):
                    a2ps = pa2.tile([128, 1024], F32, tag="a2ps",
                                    name="a2ps")
                    w2sv = w2s_t[:].rearrange("p (t c) -> p t c", c=128)
                    for m in range(7):
                        # k-tile pair [e chunk; zeros at col 896]
                        lhsT = _sv(e_t[:], m * 128,
                                   [(896 - m * 128, 2), (1, 128)])
                        nc.tensor.matmul(
                            a2ps[:, m * 128:(m + 1) * 128],
                            lhsT=lhsT, rhs=w2sv,
                            start=True, stop=True, perf_mode=DR)
                    src = a2ps[:, 0:896].rearrange("p (m c) -> p m c",
                                                   c=128)
                    dst = a2tv[:, :, gi, :]
                    flex(896,
                         lambda: nc.scalar.copy(dst, src),
                         lambda: nc.vector.tensor_scalar_add(dst, src, 0.0))

                for gi in range(NG):
                    if gi % 8 == 2 and gi // 8 + 2 <= (NG - 1) // 8:
                        fetch_ab(gi // 8 + 2)
                    ab4 = abtiles[gi // 8]
                    j = gi % 8

                    lg = pbig.tile([128, 1024], F32, tag="big", name="big")
                    for ci in range(2):
                        pchunk = lg[:, PSC[ci]]
                        nc.tensor.matmul(
                            pchunk, lhsT=q2tv[:, :, gi * 128:(gi + 1) * 128],
                            rhs=k2v[:, :, CH[ci]],
                            start=True, stop=False, perf_mode=DR)
                        abrhs = _sv(ab4[:], j * N + ci * 392,
                                    [(392, 2), (1, 392)])
                        nc.tensor.matmul(
                            pchunk,
                            lhsT=idz_t[:].rearrange("p (t c) -> p t c",
                                                    c=128),
                            rhs=abrhs,
                            start=False, stop=True, perf_mode=DR)

                    e_t = cw.tile([128, 1024], FP8, tag="e", name="e",
                                  bufs=4)
                    if gi < 4:
                        nc.gpsimd.memset(e_t[:, 784:1024], 0.0)
                    z = cz.tile([128, 1], F32, tag="z", name="z")
                    ev = e_t[:, 0:784].rearrange("p (a c) -> p a c", c=392)
                    act_only(784, lambda: nc.scalar.activation(
                        ev, psum2view(lg), AF.Exp, bias=bth1_t,
                        accum_out=z[:]), accum=True)

                    w2s_t = cw.tile([128, 256], FP8, tag="w2s", name="w2s",
                                    bufs=4)
                    if gi < 4:
                        nc.gpsimd.memset(w2s_t[:, 128:256], 0.0)
                    r = cz.tile([128, 1], F32, tag="r", name="r")
                    nc.vector.reciprocal(r[:], z[:])
                    dve_only(128, lambda: nc.vector.tensor_scalar_mul(
                        w2s_t[:, 0:128], w2bd_t[:], r[:]))

                    pending.append((gi, e_t, w2s_t))
                    if len(pending) > 2:
                        th2t_and_copy(*pending.pop(0))

                    # fillers
                    if gi < 28:
                        if tasks_early and (gi * 15) // 28 > \
                                14 - len(tasks_early):
                            tasks_early.pop(0)()
                    elif gi >= 30 and tasks_late and (gi - 30) % 2 == 0:
                        tasks_late.pop(0)()
                    for _ in range(2):
                        if q2_ops:
                            emit_q2(*q2_ops.pop(0))

                while pending:
                    th2t_and_copy(*pending.pop(0))
                while tasks_early:
                    tasks_early.pop(0)()
                while tasks_late:
                    tasks_late.pop(0)()

            # ================= phase D tail ==========================
            with (
                tc.tile_pool(name="pd", bufs=2, space="PSUM") as pd,
                tc.tile_pool(name="pe", bufs=1, space="PSUM") as pe,
                tc.tile_pool(name="ow", bufs=1) as ow,
            ):
                nc.sync.dma_start(wp_w[:], wp8.ap()[:])
                ot = [ow.tile([128, N], F32, tag=f"ot{mt}", name=f"ot{mt}")
                      for mt in range(3)]

                # chunk-0 p-projection first (all osum chunk-0 ready)
                pp0 = [pe.tile([128, 448], F32, tag=f"pp0{mt}",
                               name=f"pp0{mt}") for mt in range(3)]
                for g in range(8):
                    for mt in range(3):
                        nc.tensor.matmul(
                            pp0[mt][:],
                            lhsT=wp_t[g][:, mt * 128:(mt + 1) * 128],
                            rhs=osum[g][:, 0:448],
                            start=(g == 0), stop=(g == 7))
                for mt in range(3):
                    flex(448, lambda m=mt: nc.scalar.activation(
                        ot[m][:, 0:448], pp0[m][:], AF.Identity,
                        bias=bp_t[m]),
                        lambda m=mt: nc.vector.tensor_scalar_add(
                            ot[m][:, 0:448], pp0[m][:], bp_t[m]))
                    nc.sync.dma_start(out.ap()[mt * 128:(mt + 1) * 128,
                                               0:448], ot[mt][:, 0:448])

                # D1 heads with chunk-1 p-projection pipelined one head back
                pp1 = [pe.tile([128, 336], F32, tag=f"pp1{mt}",
                               name=f"pp1{mt}") for mt in range(3)]

                def pp1_g(g):
                    for mt in range(3):
                        nc.tensor.matmul(
                            pp1[mt][:],
                            lhsT=wp_t[g][:, mt * 128:(mt + 1) * 128],
                            rhs=osum[g][:, 448:784],
                            start=(g == 0), stop=(g == 7))

                for g in range(8):
                    dw_attnv(g, 1, pd, "po2", tilew=448)
                    if g >= 1:
                        pp1_g(g - 1)
                pp1_g(7)
                for mt in range(3):
                    if mt == 1:
                        nc.vector.tensor_scalar_add(
                            ot[mt][:, 448:784], pp1[mt][:], bp_t[mt])
                    else:
                        nc.scalar.activation(
                            ot[mt][:, 448:784], pp1[mt][:], AF.Identity,
                            bias=bp_t[mt])
                    trig = nc.gpsimd if mt == 1 else nc.sync
                    trig.dma_start(out.ap()[mt * 128:(mt + 1) * 128,
                                            448:784],
                                   ot[mt][:, 448:784])

    nc.compile()
    return nc


def _prep_common(inputs):
    f32 = np.float32
    scale = np.float32(KD ** -0.5)
    q_s, q_b = inputs["q_s"], inputs["q_b"]
    k_s, k_b = inputs["k_s"], inputs["k_b"]
    v_s, v_b = inputs["v_s"], inputs["v_b"]
    p_s, p_b = inputs["p_s"], inputs["p_b"]

    Wq = np.asarray(inputs["Wq"], f32) * np.asarray(q_s, f32)[:, None] * scale
    bqv = (np.asarray(q_s, f32) * np.asarray(inputs["bq"], f32)
           + np.asarray(q_b, f32)) * scale
    Wk = np.asarray(inputs["Wk"], f32) * np.asarray(k_s, f32)[:, None]
    bkv = np.asarray(k_s, f32) * np.asarray(inputs["bk"], f32) \
        + np.asarray(k_b, f32)
    Wv = np.asarray(inputs["Wv"], f32) * np.asarray(v_s, f32)[:, None]
    bvv = np.asarray(v_s, f32) * np.asarray(inputs["bv"], f32) \
        + np.asarray(v_b, f32)
    Wp = np.asarray(inputs["Wp"], f32) * np.asarray(p_s, f32)[:, None]
    bpv = np.asarray(p_s, f32) * np.asarray(inputs["bp"], f32) \
        + np.asarray(p_b, f32)

    Wth1 = np.asarray(inputs["Wth1"], f32)
    bth1 = np.asarray(inputs["bth1"], f32)
    Wth2 = np.asarray(inputs["Wth2"], f32)
    bth2 = np.asarray(inputs["bth2"], f32)

    # th1-folded rel-pos bias table, rows (group, g, i)
    ab1 = Wth1 @ np.asarray(inputs["attention_biases"], f32)
    idx = np.asarray(inputs["bias_idxs"])
    ab_full = ab1[:, idx]                                  # [8,784,784]
    # rows (group, g, i) -> partition-major [128, NG*784] so each 4-group
    # fetch is one contiguous segment per partition
    abt = np.ascontiguousarray(
        ab_full.reshape(8, NG, 16, N).transpose(1, 0, 2, 3)
    ).reshape(NG, 128, N).transpose(1, 0, 2).reshape(128, NG * N)
    abt = np.ascontiguousarray(abt).astype(F8)

    # depthwise weights folded with BN -> fp8 diagonal pair blocks
    wvl = np.asarray(inputs["Wvl"], f32)[:, 0, :, :].reshape(DH, 9)
    vl_s = np.asarray(inputs["vl_s"], f32)
    wtap = wvl * vl_s[:, None]                             # [1024, 9]
    bdw = (np.asarray(inputs["bvl"], f32) * vl_s
           + np.asarray(inputs["vl_b"], f32))
    dwp = np.zeros((128, 8 * 9 * 128), f32)
    for g in range(8):
        for t in range(9):
            blk = np.zeros((128, 128), f32)
            np.fill_diagonal(blk, wtap[g * 128:(g + 1) * 128, t])
            dwp[:, g * 1152 + t * 128:g * 1152 + (t + 1) * 128] = blk

    def ktile_pack(wT, nk):
        C = wT.shape[1]
        return np.ascontiguousarray(
            wT.reshape(nk, 128, C).transpose(1, 0, 2).reshape(128, nk * C))

    sqv = np.repeat(Wth1.T, KD, axis=0).astype(f32)        # [256, 8]
    vecs = np.zeros((128, 48), f32)
    vecs[:, 0:2] = bqv.reshape(2, 128).T
    vecs[:, 2:4] = bkv.reshape(2, 128).T
    vecs[:, 4:12] = bvv.reshape(8, 128).T
    s2p = Wth2.sum(axis=1)                                 # no N*bth2 term
    bdw2 = bdw + bvv * np.repeat(s2p, D)
    vecs[:, 12:20] = bdw2.reshape(8, 128).T
    vecs[:, 20:23] = bpv.reshape(3, 128).T
    vecs[:, 23] = np.repeat(bth1, 16)
    for g in range(8):
        vecs[:, 24 + g] = bth2[g]
    vecs[:, 32:40] = sqv[0:128]
    vecs[:, 40:48] = sqv[128:256]

    common = {
        "wq3": ktile_pack(np.ascontiguousarray(Wq.T), 3).astype(BF),
        "wk3": ktile_pack(np.ascontiguousarray(Wk.T), 3).astype(BF),
        "wv3": ktile_pack(np.ascontiguousarray(Wv.T), 3).astype(BF),
        "wp8": ktile_pack(np.ascontiguousarray(Wp.T), 8).astype(BF),
        "vecs": vecs,
        "w2bd": np.kron(Wth2.T, np.eye(16, dtype=f32)).astype(f32),
        "idz": np.concatenate([np.eye(128, dtype=f32),
                               np.zeros((128, 128), f32)],
                              axis=1).astype(F8),
        "dwp": dwp.astype(F8),
        "abt": abt,
    }
    return common


def kernel(**inputs):
    global LAST_RESULTS
    if "nc" not in _CACHE:
        _CACHE["nc"] = _build_program()
    nc = _CACHE["nc"]

    common = _prep_common(inputs)
    x = np.asarray(inputs["x"], np.float32)          # [8, 384, 28, 28]
    in_maps = []
    for c in range(B):
        m = dict(common)
        xc = x[c].reshape(3, 128, N).transpose(1, 0, 2).reshape(128, 3 * N)
        m["x_c"] = np.ascontiguousarray(xc).astype(BF)
        in_maps.append(m)

    import os
    trace = bool(int(os.environ.get("KERNEL_TRACE", "0")))
    res = run_bass_kernel_spmd(nc, in_maps, core_ids=list(range(B)),
                               trace=trace)
    LAST_RESULTS = res
    out = np.stack([res.results[c]["out"].reshape(DIM, RES, RES)
                    for c in range(B)])
    return out.astype(np.float32)
